# revision 19
# baseline (speedup 1.0000x reference)
"""CG-SENSE MRI reconstruction (nn_CGClass) on 8 Trainium2 NeuronCores.

Strategy: data-parallel over batch (B=8 -> 1 sample per core). Per core the
whole CG (10 iterations) runs on-chip. fft2/ifft2 are dense 320-point DFTs
done on the tensor engine as fp16 matmuls via the transpose-free primitive
OUT = Z^T @ A (data stationary, DFT matrix moving); applying it twice gives
F @ Z @ F with no transposes.

v3 speedups over the first working version:
- Mirror symmetry of the DFT matrix (cos even / sin odd in the output
  column) halves PE work on sides S1/S3: only columns 0..160 of the four
  real products are computed; both halves of the spectrum are assembled
  with +/- combines (half on DVE, half on Pool) using reversed-stride APs.
- The ragged K=64 row-block (rows 256:320) is packed: the imag tile's
  block 2 is DMA'd into the real tile's partitions 64:127 so one K=128
  matmul with a host-stacked [table_r; table_i] moving operand covers both
  contraction tails (sides S2/S4).
- The coil sum ap = sum_c conj(s_c) * ifft(...) accumulates in PSUM via
  identity matmuls instead of DVE read-modify-write adds.
- CG state updates use the fused affine_then_add DVE op (1 instr instead
  of 2), alpha/beta broadcasts use gpsimd partition_broadcast (no PSUM).
- The last CG iteration only computes x (r/p/rTr updates skipped).

Layout: each 320x320 real array lives in SBUF as [128, 960]: free-dim block
t in {0,1,2} holds image rows [128t : 128t+{128,128,64}]. Block 2 uses
partitions 0..63; pad regions are kept zero (NaN hygiene for reductions),
except real-part stationary tiles whose partitions 64:127 of block 2 hold
the imag tile's block 2 (the K-packing above).
"""
import os
from contextlib import ExitStack

import numpy as np

import concourse.bass as bass  # noqa: F401
import concourse.tile as tile
from concourse import mybir, bass_utils, bacc

F32 = mybir.dt.float32
F16 = mybir.dt.float16
CP = None  # set later (ActivationFunctionType.Copy)

H = 320
B, C = 8, 12
N_ITER = int(os.environ.get("CG_ITERS", "10"))
KT = [(0, 128), (128, 128), (256, 64)]  # (row_start, rows) per block
ML = 161  # mirror low-half width (columns 0..160)

_PROGRAM = None
TRACE = bool(os.environ.get("CG_TRACE"))


def _mblk(t):
    return slice(320 * t, 320 * t + 320)


def _mirb(t):
    return slice(ML * t, ML * t + ML)


def _dft_mats():
    j = np.arange(H)
    ang = -2.0 * np.pi * np.outer(j, j) / H
    scale = 1.0 / np.sqrt(H)
    Fr = (np.cos(ang) * scale).astype(np.float32)
    Fi = (np.sin(ang) * scale).astype(np.float32)
    return Fr, Fi


def _blocks(a, w):
    out = np.zeros((128, 3 * w), np.float32)
    for t, (s, sz) in enumerate(KT):
        out[:sz, w * t:w * t + w] = a[s:s + sz, :w]
    return out


def _build_consts():
    Fr, Fi = _dft_mats()

    def pk(ta, tb):
        out = np.zeros((128, 320), np.float32)
        out[0:64] = ta[256:320]
        out[64:128] = tb[256:320]
        return out

    ident = np.eye(128, dtype=np.float16)
    return {
        "c_fr": _blocks(Fr, 320).astype(np.float16),
        "c_fi": _blocks(Fi, 320).astype(np.float16),
        "c_nfi": _blocks(-Fi, 320).astype(np.float16),
        "pk_f_re": pk(Fr, -Fi).astype(np.float16),
        "pk_f_im": pk(Fi, Fr).astype(np.float16),
        "pk_g_re": pk(Fr, Fi).astype(np.float16),
        "pk_g_im": pk(-Fi, Fr).astype(np.float16),
        "m_frfi": _blocks(np.concatenate([Fr[:, :ML], Fi[:, :ML]], axis=1),
                          2 * ML).astype(np.float16),
        "m_fifr": _blocks(np.concatenate([Fi[:, :ML], Fr[:, :ML]], axis=1),
                          2 * ML).astype(np.float16),
        "ident": ident,
        "nident": (-ident),
        "ones_col": np.ones((128, 1), np.float32),
    }


def _selfcheck():
    """Host numpy emulation of the mirror/packed table math vs np.fft."""
    rng = np.random.default_rng(0)
    zr = rng.standard_normal((H, H)).astype(np.float32)
    zi = rng.standard_normal((H, H)).astype(np.float32)
    Fr, Fi = _dft_mats()

    def side_full(zr, zi, chain):
        # psum_re/im = sum over products; emulates emit_full incl kt packing
        if chain == "F":
            re = zr.T @ Fr + zi.T @ (-Fi)
            im = zr.T @ Fi + zi.T @ Fr
        else:
            re = zr.T @ Fr + zi.T @ Fi
            im = zr.T @ (-Fi) + zi.T @ Fr
        return re, im

    def side_mirror(zr, zi, chain):
        A = zr.T @ Fr[:, :ML]
        Dp = zr.T @ Fi[:, :ML]
        Bp = zi.T @ Fi[:, :ML]
        Cp = zi.T @ Fr[:, :ML]
        re = np.zeros((H, H), np.float32)
        im = np.zeros((H, H), np.float32)
        if chain == "F":
            re[:, :ML] = A - Bp
            re[:, ML:] = (A + Bp)[:, 159:0:-1]
            im[:, :ML] = Cp + Dp
            im[:, ML:] = (Cp - Dp)[:, 159:0:-1]
        else:
            re[:, :ML] = A + Bp
            re[:, ML:] = (A - Bp)[:, 159:0:-1]
            im[:, :ML] = Cp - Dp
            im[:, ML:] = (Cp + Dp)[:, 159:0:-1]
        return re, im

    z = zr + 1j * zi
    # forward fft2 = S1 (mirror, F) then S2 (full, F)
    s1r, s1i = side_mirror(zr, zi, "F")
    s2r, s2i = side_full(s1r, s1i, "F")
    want = np.fft.fft2(z, norm="ortho")
    err = np.abs((s2r + 1j * s2i) - want).max()
    assert err < 1e-3, f"fwd mirror pipeline err {err}"
    # inverse = S3 (mirror, G) then S4 (full, G)
    s3r, s3i = side_mirror(s2r, s2i, "G")
    s4r, s4i = side_full(s3r, s3i, "G")
    err2 = np.abs((s4r + 1j * s4i) - z).max()
    assert err2 < 1e-3, f"roundtrip err {err2}"


def _build_program():
    nc = bacc.Bacc("TRN2", target_bir_lowering=False, debug=False)
    ACT_CP = mybir.ActivationFunctionType.Copy
    ACT_SQ = mybir.ActivationFunctionType.Square

    d = {}
    d["x_re"] = nc.dram_tensor("x_re", [H, H], F32, kind="ExternalInput")
    d["x_im"] = nc.dram_tensor("x_im", [H, H], F32, kind="ExternalInput")
    d["y_re"] = nc.dram_tensor("y_re", [C, H, H], F32, kind="ExternalInput")
    d["y_im"] = nc.dram_tensor("y_im", [C, H, H], F32, kind="ExternalInput")
    d["s_re"] = nc.dram_tensor("s_re", [C, H, H], F16, kind="ExternalInput")
    d["s_im"] = nc.dram_tensor("s_im", [C, H, H], F16, kind="ExternalInput")
    d["mask"] = nc.dram_tensor("mask", [H, H], F32, kind="ExternalInput")
    d["lam_b"] = nc.dram_tensor("lam_b", [128, 1], F32, kind="ExternalInput")
    for nm, shp, dt in [
        ("c_fr", [128, 960], F16), ("c_fi", [128, 960], F16),
        ("c_nfi", [128, 960], F16), ("pk_f_re", [128, 320], F16),
        ("pk_f_im", [128, 320], F16), ("pk_g_re", [128, 320], F16),
        ("pk_g_im", [128, 320], F16), ("m_frfi", [128, 966], F16),
        ("m_fifr", [128, 966], F16), ("ident", [128, 128], F16),
        ("nident", [128, 128], F16), ("ones_col", [128, 1], F32),
    ]:
        d[nm] = nc.dram_tensor(nm, shp, dt, kind="ExternalInput")
    d["out"] = nc.dram_tensor("out", [2, H, H], F32, kind="ExternalOutput")
    _dbg = os.environ.get("CG_DEBUG", "")
    if _dbg == "stage":
        d["dbg16"] = nc.dram_tensor("dbg16", [2, H, H], F16, kind="ExternalOutput")
        d["dbg32"] = nc.dram_tensor("dbg32", [2, H, H], F32, kind="ExternalOutput")

    with tile.TileContext(nc) as tc, ExitStack() as ctx:
        persist = ctx.enter_context(tc.tile_pool(name="persist", bufs=1))
        stg16 = ctx.enter_context(tc.tile_pool(name="stg16", bufs=2))
        tmp32 = ctx.enter_context(tc.tile_pool(name="tmp32", bufs=2))
        ps = ctx.enter_context(tc.tile_pool(name="ps", bufs=2, space="PSUM"))
        aps = ctx.enter_context(tc.tile_pool(name="aps", bufs=1, space="PSUM"))

        _asm_hi = nc.vector if os.environ.get("CG_ASM") == "dve" else nc.gpsimd

        def load_blocks(dst, src_ap):
            for t, (s, sz) in enumerate(KT):
                nc.sync.dma_start(dst[0:sz, _mblk(t)], src_ap[s:s + sz, :])

        def zero_pad(t32, eng=None):
            (eng or nc.vector).memset(t32[64:128, 640:960], 0.0)

        # ---- persistent tiles ----
        sr = [persist.tile([128, 960], F16, tag=f"sr{c}", name=f"sr{c}") for c in range(C)]
        si = [persist.tile([128, 960], F16, tag=f"si{c}", name=f"si{c}") for c in range(C)]
        mask2 = persist.tile([128, 960], F32, tag="mask2", name="mask2")
        mask1 = persist.tile([128, 960], F32, tag="mask1", name="mask1")
        p_re = persist.tile([128, 960], F32, tag="p_re", name="p_re")
        p_im = persist.tile([128, 960], F32, tag="p_im", name="p_im")
        r_re = persist.tile([128, 960], F32, tag="r_re", name="r_re")
        r_im = persist.tile([128, 960], F32, tag="r_im", name="r_im")
        x_re = persist.tile([128, 960], F32, tag="x_re", name="x_re")
        x_im = persist.tile([128, 960], F32, tag="x_im", name="x_im")
        fr = persist.tile([128, 960], F16, tag="fr", name="fr")
        fi = persist.tile([128, 960], F16, tag="fi", name="fi")
        nfi = persist.tile([128, 960], F16, tag="nfi", name="nfi")
        pk_f_re = persist.tile([128, 320], F16, tag="pk_f_re", name="pk_f_re")
        pk_f_im = persist.tile([128, 320], F16, tag="pk_f_im", name="pk_f_im")
        pk_g_re = persist.tile([128, 320], F16, tag="pk_g_re", name="pk_g_re")
        pk_g_im = persist.tile([128, 320], F16, tag="pk_g_im", name="pk_g_im")
        m_frfi = persist.tile([128, 966], F16, tag="m_frfi", name="m_frfi")
        m_fifr = persist.tile([128, 966], F16, tag="m_fifr", name="m_fifr")
        ident = persist.tile([128, 128], F16, tag="ident", name="ident")
        nident = persist.tile([128, 128], F16, tag="nident", name="nident")
        ones_col = persist.tile([128, 1], F32, tag="ones_col", name="ones_col")
        lam_b = persist.tile([128, 1], F32, tag="lam_b", name="lam_b")
        alpha_b = persist.tile([128, 1], F32, tag="alpha_b", name="alpha_b")
        nalpha_b = persist.tile([128, 1], F32, tag="nalpha_b", name="nalpha_b")
        beta_b = persist.tile([128, 1], F32, tag="beta_b", name="beta_b")
        dacc = persist.tile([128, 2], F32, tag="dacc", name="dacc")
        sc = persist.tile([1, 12], F32, tag="sc", name="sc")
        # sc slots: 0=rTr, 1=inv_rTr, 2=pAp, 3=alpha, 4=rTrNew, 5=beta,
        # 6,7=tmp, 8=nalpha
        scr = persist.tile([128, 960], F32, tag="scr", name="scr")
        scr2 = persist.tile([128, 960], F32, tag="scr2", name="scr2")
        jnk = persist.tile([128, 960], F32, tag="jnk", name="jnk")
        lp16_re = persist.tile([128, 960], F16, tag="lp16_re", name="lp16_re")
        lp16_im = persist.tile([128, 960], F16, tag="lp16_im", name="lp16_im")

        def ap_tiles():
            return (aps.tile([128, 480], F32, tag="apre0", name="apre0"),
                    aps.tile([128, 480], F32, tag="apre1", name="apre1"),
                    aps.tile([128, 480], F32, tag="apim0", name="apim0"),
                    aps.tile([128, 480], F32, tag="apim1", name="apim1"))

        def emit_full(zr16, zi16, chain, consume):
            """Full-width side with packed-K tail. psum(re,im) per m-block.

            zr16's partitions 64:127 @ cols 640:960 must hold zi16's block 2
            (the dup DMA every producer does)."""
            if chain == "F":
                tB, tC = fi, nfi
                pkre, pkim = pk_f_re, pk_f_im
            else:
                tB, tC = nfi, fi
                pkre, pkim = pk_g_re, pk_g_im
            for mt, (ms, msz) in enumerate(KT):
                P = ps.tile([128, 322], F32, tag="P", name="P")
                Q = ps.tile([128, 322], F32, tag="Q", name="Q")
                pre = P[0:msz, 0:320]
                pim = Q[0:msz, 0:320]
                zr0 = zr16[0:128, 128 * mt:128 * mt + msz]
                zr1 = zr16[0:128, 320 + 128 * mt:320 + 128 * mt + msz]
                zpk = zr16[0:128, 640 + 128 * mt:640 + 128 * mt + msz]
                zi0 = zi16[0:128, 128 * mt:128 * mt + msz]
                zi1 = zi16[0:128, 320 + 128 * mt:320 + 128 * mt + msz]
                mm = nc.tensor.matmul
                mm(pre, zr0, fr[:, _mblk(0)], start=True, stop=False)
                mm(pim, zr0, tB[:, _mblk(0)], start=True, stop=False)
                mm(pre, zr1, fr[:, _mblk(1)], start=False, stop=False)
                mm(pim, zr1, tB[:, _mblk(1)], start=False, stop=False)
                mm(pre, zpk, pkre[:, 0:320], start=False, stop=False)
                mm(pim, zpk, pkim[:, 0:320], start=False, stop=False)
                mm(pre, zi0, tC[:, _mblk(0)], start=False, stop=False)
                mm(pim, zi0, fr[:, _mblk(0)], start=False, stop=False)
                mm(pre, zi1, tC[:, _mblk(1)], start=False, stop=True)
                mm(pim, zi1, fr[:, _mblk(1)], start=False, stop=True)
                consume(mt, msz, pre, pim)

        def emit_mirror(zr16, zi16, chain, dstr, dsti, eng_lo, eng_hi):
            """Half-spectrum side: products A=zr@FrL, D'=zr@FiL, B'=zi@FiL,
            C'=zi@FrL; +/- assembly fills both halves of dstr/dsti."""
            for mt, (ms, msz) in enumerate(KT):
                P = ps.tile([128, 322], F32, tag="P", name="P")
                Q = ps.tile([128, 322], F32, tag="Q", name="Q")
                mm = nc.tensor.matmul
                for kt, (ks, ksz) in enumerate(KT):
                    zrk = zr16[0:ksz, 320 * kt + 128 * mt:320 * kt + 128 * mt + msz]
                    mm(P[0:msz, 0:322], zrk, m_frfi[0:ksz, 322 * kt:322 * kt + 322],
                       start=(kt == 0), stop=(kt == 2))
                for kt, (ks, ksz) in enumerate(KT):
                    zik = zi16[0:ksz, 320 * kt + 128 * mt:320 * kt + 128 * mt + msz]
                    mm(Q[0:msz, 0:322], zik, m_fifr[0:ksz, 322 * kt:322 * kt + 322],
                       start=(kt == 0), stop=(kt == 2))
                # assembly: copy psums to SBUF (gpsimd can't read PSUM)
                Psb = tmp32.tile([128, 322], F32, tag="Psb", name="Psb")
                Qsb = tmp32.tile([128, 322], F32, tag="Qsb", name="Qsb")
                nc.scalar.copy(Psb[0:msz, :], P[0:msz, :])
                nc.scalar.copy(Qsb[0:msz, :], Q[0:msz, :])
                lo_r = dstr[0:msz, 320 * mt:320 * mt + ML]
                hi_r = dstr[0:msz, 320 * mt + ML:320 * mt + 320]
                lo_i = dsti[0:msz, 320 * mt:320 * mt + ML]
                hi_i = dsti[0:msz, 320 * mt + ML:320 * mt + 320]
                sbA = Psb[0:msz, 0:ML]
                sbAr = Psb[0:msz, 159:0:-1]
                sbD = Psb[0:msz, ML:322]
                sbDr = Psb[0:msz, 320:161:-1]
                qB = Qsb[0:msz, 0:ML]
                qBr = Qsb[0:msz, 159:0:-1]
                qC = Qsb[0:msz, ML:322]
                qCr = Qsb[0:msz, 320:161:-1]
                if chain == "F":
                    eng_lo.tensor_sub(lo_r, sbA, qB)          # A - B'
                    eng_hi.tensor_add(hi_r, sbAr, qBr)        # (A + B')rev
                    eng_lo.tensor_add(lo_i, sbD, qC)          # C' + D'
                    eng_hi.tensor_sub(hi_i, qCr, sbDr)        # (C' - D')rev
                else:
                    eng_lo.tensor_add(lo_r, sbA, qB)
                    eng_hi.tensor_sub(hi_r, sbAr, qBr)
                    eng_lo.tensor_sub(lo_i, qC, sbD)
                    eng_hi.tensor_add(hi_i, qCr, sbDr)
                if mt == 2:
                    nc.sync.dma_start(dstr[64:128, 640:960], dsti[0:64, 640:960])

        def cmul_to_fp16(ar, ai, br, bi, outr, outi):
            """(outr + i outi) = (ar + i ai)(br + i bi); fp16 out + dup."""
            t1 = stg16.tile([128, 960], F16, tag="mm_t1", name="mm_t1")
            t2 = stg16.tile([128, 960], F16, tag="mm_t2", name="mm_t2")
            t3 = stg16.tile([128, 960], F16, tag="mm_t3", name="mm_t3")
            t4 = stg16.tile([128, 960], F16, tag="mm_t4", name="mm_t4")
            nc.gpsimd.tensor_mul(t1[:], ar[:], br[:])
            nc.gpsimd.tensor_mul(t2[:], ai[:], bi[:])
            nc.vector.tensor_mul(t3[:], ar[:], bi[:])
            nc.vector.tensor_mul(t4[:], ai[:], br[:])
            nc.vector.tensor_sub(outr[:], t1[:], t2[:])
            nc.vector.tensor_add(outi[:], t3[:], t4[:])
            nc.sync.dma_start(outr[64:128, 640:960], outi[0:64, 640:960])

        def ifft_and_combine(c, inr16, ini16, first, last, aptiles):
            """S3 (mirror G) + S4 (full G) + conj(s_c) products + PSUM accum."""
            apre0, apre1, apim0, apim1 = aptiles
            s3r = stg16.tile([128, 960], F16, tag="s3r", name="s3r")
            s3i = stg16.tile([128, 960], F16, tag="s3i", name="s3i")
            emit_mirror(inr16, ini16, "G", s3r, s3i, nc.vector, _asm_hi)

            u4r = tmp32.tile([128, 960], F32, tag="u4r", name="u4r")
            u4i = tmp32.tile([128, 960], F32, tag="u4i", name="u4i")

            def consume4(mt, msz, pre, pim):
                nc.scalar.copy(u4r[0:msz, _mblk(mt)], pre[0:msz, :])
                nc.scalar.copy(u4i[0:msz, _mblk(mt)], pim[0:msz, :])
            emit_full(s3r, s3i, "G", consume4)
            if _dbg == "stage" and c == 0:
                for t, (s, sz) in enumerate(KT):
                    nc.sync.dma_start(d["dbg16"].ap()[0, s:s + sz, :], s3r[0:sz, _mblk(t)])
                    nc.sync.dma_start(d["dbg16"].ap()[1, s:s + sz, :], s3i[0:sz, _mblk(t)])
                    nc.sync.dma_start(d["dbg32"].ap()[0, s:s + sz, :], u4r[0:sz, _mblk(t)])
                    nc.sync.dma_start(d["dbg32"].ap()[1, s:s + sz, :], u4i[0:sz, _mblk(t)])

            # ap_re += sr*u4r + si*u4i ; ap_im += sr*u4i - si*u4r
            w1 = stg16.tile([128, 960], F16, tag="w1", name="w1")
            w2 = stg16.tile([128, 960], F16, tag="w2", name="w2")
            w3 = stg16.tile([128, 960], F16, tag="w3", name="w3")
            w4 = stg16.tile([128, 960], F16, tag="w4", name="w4")
            nc.vector.tensor_mul(w1[:], sr[c][:], u4r[:])
            nc.gpsimd.tensor_mul(w2[:], si[c][:], u4i[:])
            nc.vector.tensor_mul(w3[:], sr[c][:], u4i[:])
            nc.gpsimd.tensor_mul(w4[:], si[c][:], u4r[:])
            mm = nc.tensor.matmul
            mm(apre0[:, :], ident[:, :], w1[:, 0:480], start=first, stop=False)
            mm(apre1[:, :], ident[:, :], w1[:, 480:960], start=first, stop=False)
            mm(apim0[:, :], ident[:, :], w3[:, 0:480], start=first, stop=False)
            mm(apim1[:, :], ident[:, :], w3[:, 480:960], start=first, stop=False)
            mm(apre0[:, :], ident[:, :], w2[:, 0:480], start=False, stop=last)
            mm(apre1[:, :], ident[:, :], w2[:, 480:960], start=False, stop=last)
            mm(apim0[:, :], nident[:, :], w4[:, 0:480], start=False, stop=last)
            mm(apim1[:, :], nident[:, :], w4[:, 480:960], start=False, stop=last)

        def seed_ap(vr, vi, aptiles):
            """Start the PSUM ap accumulation group with lam * (vr, vi)."""
            apre0, apre1, apim0, apim1 = aptiles
            nc.scalar.mul(lp16_re[:], vr[:], lam_b[:, 0:1])
            nc.scalar.mul(lp16_im[:], vi[:], lam_b[:, 0:1])
            mm = nc.tensor.matmul
            mm(apre0[:, :], ident[:, :], lp16_re[:, 0:480], start=True, stop=False)
            mm(apre1[:, :], ident[:, :], lp16_re[:, 480:960], start=True, stop=False)
            mm(apim0[:, :], ident[:, :], lp16_im[:, 0:480], start=True, stop=False)
            mm(apim1[:, :], ident[:, :], lp16_im[:, 480:960], start=True, stop=False)

        def reduce_dacc(slot):
            pd = ps.tile([128, 322], F32, tag="P", name="pdot")
            nc.tensor.matmul(pd[0:1, 0:2], ones_col[:, 0:1], dacc[:, 0:2],
                             start=True, stop=True)
            nc.vector.tensor_copy(sc[0:1, 6:8], pd[0:1, 0:2])
            nc.vector.tensor_add(sc[0:1, slot:slot + 1], sc[0:1, 6:7],
                                 sc[0:1, 7:8])

        def dot_self(a_re, a_im, slot):
            nc.scalar.activation(jnk[:], a_re[:], ACT_SQ, accum_out=dacc[:, 0:1])
            nc.scalar.activation(jnk[:], a_im[:], ACT_SQ, accum_out=dacc[:, 1:2])
            reduce_dacc(slot)

        def dot_p_ap(aptiles, slot):
            apre0, apre1, apim0, apim1 = aptiles
            nc.vector.tensor_mul(scr[:, 0:480], p_re[:, 0:480], apre0[:, :])
            nc.vector.tensor_mul(scr[:, 480:960], p_re[:, 480:960], apre1[:, :])
            nc.vector.tensor_mul(scr2[:, 0:480], p_im[:, 0:480], apim0[:, :])
            nc.vector.tensor_mul(scr2[:, 480:960], p_im[:, 480:960], apim1[:, :])
            nc.scalar.activation(jnk[:], scr[:], ACT_CP, accum_out=dacc[:, 0:1])
            nc.scalar.activation(jnk[:], scr2[:], ACT_CP, accum_out=dacc[:, 1:2])
            reduce_dacc(slot)

        # ---- load constants + inputs ----
        for nm, t in [("c_fr", fr), ("c_fi", fi), ("c_nfi", nfi),
                      ("pk_f_re", pk_f_re), ("pk_f_im", pk_f_im),
                      ("pk_g_re", pk_g_re), ("pk_g_im", pk_g_im),
                      ("m_frfi", m_frfi), ("m_fifr", m_fifr), ("ident", ident),
                      ("nident", nident), ("ones_col", ones_col),
                      ("lam_b", lam_b)]:
            nc.sync.dma_start(t[:], d[nm].ap())
        for c in range(C):
            load_blocks(sr[c], d["s_re"].ap()[c])
            load_blocks(si[c], d["s_im"].ap()[c])
            zero_pad(sr[c])
            zero_pad(si[c])
        load_blocks(mask1, d["mask"].ap())
        zero_pad(mask1)
        load_blocks(x_re, d["x_re"].ap())
        load_blocks(x_im, d["x_im"].ap())
        zero_pad(x_re)
        zero_pad(x_im)
        nc.vector.tensor_mul(mask2[:], mask1[:], mask1[:])

        # ---- phase 1: rhs = AH(mask*y) + lam*x ; r0 = p0 = rhs ; x0 = 0 ----
        aptiles = ap_tiles()
        seed_ap(x_re, x_im, aptiles)

        def make_my(c):
            yr = tmp32.tile([128, 960], F32, tag="yr", name="yr")
            yi = tmp32.tile([128, 960], F32, tag="yi", name="yi")
            load_blocks(yr, d["y_re"].ap()[c])
            load_blocks(yi, d["y_im"].ap()[c])
            zero_pad(yr, nc.gpsimd)
            zero_pad(yi, nc.gpsimd)
            myr = stg16.tile([128, 960], F16, tag="spr", name="myr")
            myi = stg16.tile([128, 960], F16, tag="spi", name="myi")
            nc.vector.tensor_mul(myr[:], yr[:], mask1[:])
            nc.vector.tensor_mul(myi[:], yi[:], mask1[:])
            nc.sync.dma_start(myr[64:128, 640:960], myi[0:64, 640:960])
            return myr, myi

        my_next = make_my(0)
        for c in range(C):
            myr, myi = my_next
            if c + 1 < C:
                my_next = make_my(c + 1)
            ifft_and_combine(c, myr, myi, first=False, last=(c == C - 1),
                             aptiles=aptiles)

        # r0 = ap (psum) ; p0 = r0 ; x0 = 0
        apre0, apre1, apim0, apim1 = aptiles
        nc.scalar.copy(r_re[:, 0:480], apre0[:, :])
        nc.scalar.copy(r_re[:, 480:960], apre1[:, :])
        nc.scalar.copy(r_im[:, 0:480], apim0[:, :])
        nc.scalar.copy(r_im[:, 480:960], apim1[:, :])
        nc.scalar.copy(p_re[:], r_re[:])
        nc.scalar.copy(p_im[:], r_im[:])
        nc.vector.memset(x_re[:], 0.0)
        nc.vector.memset(x_im[:], 0.0)

        dot_self(r_re, r_im, 0)          # rTr0
        nc.vector.reciprocal(sc[0:1, 1:2], sc[0:1, 0:1])

        if _dbg in ("rhs", "stage"):
            nc.scalar.copy(x_re[:], r_re[:])
            nc.scalar.copy(x_im[:], r_im[:])

        # ---- phase 2: CG iterations ----
        def cg_iteration(trim):
            aptiles = ap_tiles()
            spr0 = stg16.tile([128, 960], F16, tag="spr", name="spr")
            spi0 = stg16.tile([128, 960], F16, tag="spi", name="spi")
            cmul_to_fp16(sr[0], si[0], p_re, p_im, spr0, spi0)
            seed_ap(p_re, p_im, aptiles)
            sp_next = (spr0, spi0)
            for c in range(C):
                spr, spi = sp_next
                s1r = stg16.tile([128, 960], F16, tag="s1r", name="s1r")
                s1i = stg16.tile([128, 960], F16, tag="s1i", name="s1i")
                emit_mirror(spr, spi, "F", s1r, s1i, nc.vector, _asm_hi)

                wr = stg16.tile([128, 960], F16, tag="wr", name="wr")
                wi = stg16.tile([128, 960], F16, tag="wi", name="wi")

                def consume2(mt, msz, pre, pim):
                    nc.vector.tensor_mul(wr[0:msz, _mblk(mt)], pre[0:msz, :],
                                         mask2[0:msz, _mblk(mt)])
                    nc.vector.tensor_mul(wi[0:msz, _mblk(mt)], pim[0:msz, :],
                                         mask2[0:msz, _mblk(mt)])
                    if mt == 2:
                        nc.sync.dma_start(wr[64:128, 640:960],
                                          wi[0:64, 640:960])
                emit_full(s1r, s1i, "F", consume2)

                if c + 1 < C:
                    sp_next = stg16.tile([128, 960], F16, tag="spr", name="spr"), \
                        stg16.tile([128, 960], F16, tag="spi", name="spi")
                    cmul_to_fp16(sr[c + 1], si[c + 1], p_re, p_im,
                                 sp_next[0], sp_next[1])
                ifft_and_combine(c, wr, wi, first=False, last=(c == C - 1),
                                 aptiles=aptiles)

            dot_p_ap(aptiles, 2)                                        # pAp
            nc.vector.reciprocal(sc[0:1, 6:7], sc[0:1, 2:3])
            nc.vector.tensor_mul(sc[0:1, 3:4], sc[0:1, 0:1], sc[0:1, 6:7])
            nc.vector.tensor_scalar_mul(sc[0:1, 8:9], sc[0:1, 3:4], -1.0)
            nc.gpsimd.partition_broadcast(alpha_b[:, 0:1], sc[0:1, 3:4])
            apre0, apre1, apim0, apim1 = aptiles

            # x += alpha p
            nc.vector.affine_then_add(x_re[:], p_re[:], x_re[:],
                                      scale=alpha_b[:, 0:1], bias=0.0)
            nc.vector.affine_then_add(x_im[:], p_im[:], x_im[:],
                                      scale=alpha_b[:, 0:1], bias=0.0)
            if trim:
                return
            nc.gpsimd.partition_broadcast(nalpha_b[:, 0:1], sc[0:1, 8:9])
            # r -= alpha Ap
            nc.vector.affine_then_add(r_re[:, 0:480], apre0[:, :],
                                      r_re[:, 0:480], scale=nalpha_b[:, 0:1],
                                      bias=0.0)
            nc.vector.affine_then_add(r_re[:, 480:960], apre1[:, :],
                                      r_re[:, 480:960], scale=nalpha_b[:, 0:1],
                                      bias=0.0)
            nc.vector.affine_then_add(r_im[:, 0:480], apim0[:, :],
                                      r_im[:, 0:480], scale=nalpha_b[:, 0:1],
                                      bias=0.0)
            nc.vector.affine_then_add(r_im[:, 480:960], apim1[:, :],
                                      r_im[:, 480:960], scale=nalpha_b[:, 0:1],
                                      bias=0.0)

            dot_self(r_re, r_im, 4)                                     # rTrNew
            nc.vector.tensor_mul(sc[0:1, 5:6], sc[0:1, 4:5], sc[0:1, 1:2])
            nc.vector.tensor_copy(sc[0:1, 0:1], sc[0:1, 4:5])
            nc.vector.reciprocal(sc[0:1, 1:2], sc[0:1, 4:5])
            nc.gpsimd.partition_broadcast(beta_b[:, 0:1], sc[0:1, 5:6])
            # p = beta p + r
            nc.vector.affine_then_add(p_re[:], p_re[:], r_re[:],
                                      scale=beta_b[:, 0:1], bias=0.0)
            nc.vector.affine_then_add(p_im[:], p_im[:], r_im[:],
                                      scale=beta_b[:, 0:1], bias=0.0)

        if _dbg not in ("rhs", "stage"):
            if N_ITER > 1:
                with tc.For_i(0, N_ITER - 1, 1):
                    cg_iteration(trim=False)
            cg_iteration(trim=True)

        for t, (s, sz) in enumerate(KT):
            nc.sync.dma_start(d["out"].ap()[0, s:s + sz, :], x_re[0:sz, _mblk(t)])
            nc.sync.dma_start(d["out"].ap()[1, s:s + sz, :], x_im[0:sz, _mblk(t)])

    nc.compile()
    return nc


def kernel(lambdaa, x_re, x_im, y_re, y_im, smaps_re, smaps_im, mask):
    global _PROGRAM
    lambdaa = np.asarray(lambdaa, np.float32)
    arrs = {
        "x_re": x_re, "x_im": x_im, "y_re": y_re, "y_im": y_im,
    }
    arrs = {k: np.ascontiguousarray(np.asarray(v, np.float32))
            for k, v in arrs.items()}
    arrs["s_re"] = np.ascontiguousarray(np.asarray(smaps_re, np.float16))
    arrs["s_im"] = np.ascontiguousarray(np.asarray(smaps_im, np.float16))
    mask = np.ascontiguousarray(np.asarray(mask, np.float32))

    _selfcheck()
    if _PROGRAM is None:
        _PROGRAM = _build_program()
    nc = _PROGRAM

    consts = _build_consts()
    lam_b = np.full((128, 1), float(lambdaa[0]), np.float32)
    in_maps = []
    for i in range(B):
        in_maps.append({
            **{k: v[i] for k, v in arrs.items()},
            "mask": np.ascontiguousarray(mask[i, 0]),
            "lam_b": lam_b,
            **consts,
        })

    res = bass_utils.run_bass_kernel_spmd(nc, in_maps, core_ids=list(range(B)),
                                          trace=TRACE)
    kernel._last_result = res
    out = np.empty((B, H, H, 2), np.float32)
    for i in range(B):
        o = res.results[i]["out"]
        out[i, :, :, 0] = o[0]
        out[i, :, :, 1] = o[1]
    return out


# revision 22
# speedup vs baseline: 1.0352x; 1.0352x over previous
"""CG-SENSE MRI reconstruction (nn_CGClass) on 8 Trainium2 NeuronCores.

Strategy: data-parallel over batch (B=8 -> 1 sample per core). Per core the
whole CG (10 iterations) runs on-chip. fft2/ifft2 are dense 320-point DFTs
done on the tensor engine as fp16 matmuls via the transpose-free primitive
OUT = Z^T @ A (data stationary, DFT matrix moving); applying it twice gives
F @ Z @ F with no transposes.

v3 speedups over the first working version:
- Mirror symmetry of the DFT matrix (cos even / sin odd in the output
  column) halves PE work on sides S1/S3: only columns 0..160 of the four
  real products are computed; both halves of the spectrum are assembled
  with +/- combines (half on DVE, half on Pool) using reversed-stride APs.
- The ragged K=64 row-block (rows 256:320) is packed: the imag tile's
  block 2 is DMA'd into the real tile's partitions 64:127 so one K=128
  matmul with a host-stacked [table_r; table_i] moving operand covers both
  contraction tails (sides S2/S4).
- The coil sum ap = sum_c conj(s_c) * ifft(...) accumulates in PSUM via
  identity matmuls instead of DVE read-modify-write adds.
- CG state updates use the fused affine_then_add DVE op (1 instr instead
  of 2), alpha/beta broadcasts use gpsimd partition_broadcast (no PSUM).
- The last CG iteration only computes x (r/p/rTr updates skipped).

Layout: each 320x320 real array lives in SBUF as [128, 960]: free-dim block
t in {0,1,2} holds image rows [128t : 128t+{128,128,64}]. Block 2 uses
partitions 0..63; pad regions are kept zero (NaN hygiene for reductions),
except real-part stationary tiles whose partitions 64:127 of block 2 hold
the imag tile's block 2 (the K-packing above).
"""
import os
from contextlib import ExitStack

import numpy as np

import concourse.bass as bass  # noqa: F401
import concourse.tile as tile
from concourse import mybir, bass_utils, bacc

F32 = mybir.dt.float32
F16 = mybir.dt.float16
CP = None  # set later (ActivationFunctionType.Copy)

H = 320
B, C = 8, 12
N_ITER = int(os.environ.get("CG_ITERS", "10"))
KT = [(0, 128), (128, 128), (256, 64)]  # (row_start, rows) per block
ML = 161  # mirror low-half width (columns 0..160)

_PROGRAM = None
TRACE = bool(os.environ.get("CG_TRACE"))


def _mblk(t):
    return slice(320 * t, 320 * t + 320)


def _mirb(t):
    return slice(ML * t, ML * t + ML)


def _dft_mats():
    j = np.arange(H)
    ang = -2.0 * np.pi * np.outer(j, j) / H
    scale = 1.0 / np.sqrt(H)
    Fr = (np.cos(ang) * scale).astype(np.float32)
    Fi = (np.sin(ang) * scale).astype(np.float32)
    return Fr, Fi


def _blocks(a, w):
    out = np.zeros((128, 3 * w), np.float32)
    for t, (s, sz) in enumerate(KT):
        out[:sz, w * t:w * t + w] = a[s:s + sz, :w]
    return out


def _build_consts():
    Fr, Fi = _dft_mats()

    def pk(ta, tb):
        out = np.zeros((128, 320), np.float32)
        out[0:64] = ta[256:320]
        out[64:128] = tb[256:320]
        return out

    ident = np.eye(128, dtype=np.float16)
    return {
        "c_fr": _blocks(Fr, 320).astype(np.float16),
        "c_fi": _blocks(Fi, 320).astype(np.float16),
        "c_nfi": _blocks(-Fi, 320).astype(np.float16),
        "pk_f_re": pk(Fr, -Fi).astype(np.float16),
        "pk_f_im": pk(Fi, Fr).astype(np.float16),
        "pk_g_re": pk(Fr, Fi).astype(np.float16),
        "pk_g_im": pk(-Fi, Fr).astype(np.float16),
        "m_frfi": _blocks(np.concatenate([Fr[:, :ML], Fi[:, :ML]], axis=1),
                          2 * ML).astype(np.float16),
        "m_fifr": _blocks(np.concatenate([Fi[:, :ML], Fr[:, :ML]], axis=1),
                          2 * ML).astype(np.float16),
        "ident": ident,
        "nident": (-ident),
        "ones_col": np.ones((128, 1), np.float32),
    }


def _selfcheck():
    """Host numpy emulation of the mirror/packed table math vs np.fft."""
    rng = np.random.default_rng(0)
    zr = rng.standard_normal((H, H)).astype(np.float32)
    zi = rng.standard_normal((H, H)).astype(np.float32)
    Fr, Fi = _dft_mats()

    def side_full(zr, zi, chain):
        # psum_re/im = sum over products; emulates emit_full incl kt packing
        if chain == "F":
            re = zr.T @ Fr + zi.T @ (-Fi)
            im = zr.T @ Fi + zi.T @ Fr
        else:
            re = zr.T @ Fr + zi.T @ Fi
            im = zr.T @ (-Fi) + zi.T @ Fr
        return re, im

    def side_mirror(zr, zi, chain):
        A = zr.T @ Fr[:, :ML]
        Dp = zr.T @ Fi[:, :ML]
        Bp = zi.T @ Fi[:, :ML]
        Cp = zi.T @ Fr[:, :ML]
        re = np.zeros((H, H), np.float32)
        im = np.zeros((H, H), np.float32)
        if chain == "F":
            re[:, :ML] = A - Bp
            re[:, ML:] = (A + Bp)[:, 159:0:-1]
            im[:, :ML] = Cp + Dp
            im[:, ML:] = (Cp - Dp)[:, 159:0:-1]
        else:
            re[:, :ML] = A + Bp
            re[:, ML:] = (A - Bp)[:, 159:0:-1]
            im[:, :ML] = Cp - Dp
            im[:, ML:] = (Cp + Dp)[:, 159:0:-1]
        return re, im

    z = zr + 1j * zi
    # forward fft2 = S1 (mirror, F) then S2 (full, F)
    s1r, s1i = side_mirror(zr, zi, "F")
    s2r, s2i = side_full(s1r, s1i, "F")
    want = np.fft.fft2(z, norm="ortho")
    err = np.abs((s2r + 1j * s2i) - want).max()
    assert err < 1e-3, f"fwd mirror pipeline err {err}"
    # inverse = S3 (mirror, G) then S4 (full, G)
    s3r, s3i = side_mirror(s2r, s2i, "G")
    s4r, s4i = side_full(s3r, s3i, "G")
    err2 = np.abs((s4r + 1j * s4i) - z).max()
    assert err2 < 1e-3, f"roundtrip err {err2}"


def _build_program():
    nc = bacc.Bacc("TRN2", target_bir_lowering=False, debug=False)
    ACT_CP = mybir.ActivationFunctionType.Copy
    ACT_SQ = mybir.ActivationFunctionType.Square

    d = {}
    d["x_re"] = nc.dram_tensor("x_re", [H, H], F32, kind="ExternalInput")
    d["x_im"] = nc.dram_tensor("x_im", [H, H], F32, kind="ExternalInput")
    d["y_re"] = nc.dram_tensor("y_re", [C, H, H], F32, kind="ExternalInput")
    d["y_im"] = nc.dram_tensor("y_im", [C, H, H], F32, kind="ExternalInput")
    d["s_re"] = nc.dram_tensor("s_re", [C, H, H], F16, kind="ExternalInput")
    d["s_im"] = nc.dram_tensor("s_im", [C, H, H], F16, kind="ExternalInput")
    d["mask"] = nc.dram_tensor("mask", [H, H], F32, kind="ExternalInput")
    d["lam_b"] = nc.dram_tensor("lam_b", [128, 1], F32, kind="ExternalInput")
    for nm, shp, dt in [
        ("c_fr", [128, 960], F16), ("c_fi", [128, 960], F16),
        ("c_nfi", [128, 960], F16), ("pk_f_re", [128, 320], F16),
        ("pk_f_im", [128, 320], F16), ("pk_g_re", [128, 320], F16),
        ("pk_g_im", [128, 320], F16), ("m_frfi", [128, 966], F16),
        ("m_fifr", [128, 966], F16), ("ident", [128, 128], F16),
        ("nident", [128, 128], F16), ("ones_col", [128, 1], F32),
    ]:
        d[nm] = nc.dram_tensor(nm, shp, dt, kind="ExternalInput")
    d["out"] = nc.dram_tensor("out", [2, H, H], F32, kind="ExternalOutput")
    _dbg = os.environ.get("CG_DEBUG", "")
    if _dbg == "stage":
        d["dbg16"] = nc.dram_tensor("dbg16", [2, H, H], F16, kind="ExternalOutput")
        d["dbg32"] = nc.dram_tensor("dbg32", [2, H, H], F32, kind="ExternalOutput")

    with tile.TileContext(nc) as tc, ExitStack() as ctx:
        persist = ctx.enter_context(tc.tile_pool(name="persist", bufs=1))
        stg16 = ctx.enter_context(tc.tile_pool(name="stg16", bufs=2))
        tmp32 = ctx.enter_context(tc.tile_pool(name="tmp32", bufs=2))
        ps = ctx.enter_context(tc.tile_pool(name="ps", bufs=2, space="PSUM"))
        aps = ctx.enter_context(tc.tile_pool(name="aps", bufs=1, space="PSUM"))

        _asm_hi = nc.vector if os.environ.get("CG_ASM") == "dve" else nc.gpsimd

        def load_blocks(dst, src_ap):
            for t, (s, sz) in enumerate(KT):
                nc.sync.dma_start(dst[0:sz, _mblk(t)], src_ap[s:s + sz, :])

        def zero_pad(t32, eng=None):
            (eng or nc.vector).memset(t32[64:128, 640:960], 0.0)

        # ---- persistent tiles ----
        sr = [persist.tile([128, 960], F16, tag=f"sr{c}", name=f"sr{c}") for c in range(C)]
        si = [persist.tile([128, 960], F16, tag=f"si{c}", name=f"si{c}") for c in range(C)]
        mask2 = persist.tile([128, 960], F32, tag="mask2", name="mask2")
        mask1 = persist.tile([128, 960], F32, tag="mask1", name="mask1")
        p_re = persist.tile([128, 960], F32, tag="p_re", name="p_re")
        p_im = persist.tile([128, 960], F32, tag="p_im", name="p_im")
        r_re = persist.tile([128, 960], F32, tag="r_re", name="r_re")
        r_im = persist.tile([128, 960], F32, tag="r_im", name="r_im")
        x_re = persist.tile([128, 960], F32, tag="x_re", name="x_re")
        x_im = persist.tile([128, 960], F32, tag="x_im", name="x_im")
        fr = persist.tile([128, 960], F16, tag="fr", name="fr")
        fi = persist.tile([128, 960], F16, tag="fi", name="fi")
        nfi = persist.tile([128, 960], F16, tag="nfi", name="nfi")
        pk_f_re = persist.tile([128, 320], F16, tag="pk_f_re", name="pk_f_re")
        pk_f_im = persist.tile([128, 320], F16, tag="pk_f_im", name="pk_f_im")
        pk_g_re = persist.tile([128, 320], F16, tag="pk_g_re", name="pk_g_re")
        pk_g_im = persist.tile([128, 320], F16, tag="pk_g_im", name="pk_g_im")
        m_frfi = persist.tile([128, 966], F16, tag="m_frfi", name="m_frfi")
        m_fifr = persist.tile([128, 966], F16, tag="m_fifr", name="m_fifr")
        ident = persist.tile([128, 128], F16, tag="ident", name="ident")
        nident = persist.tile([128, 128], F16, tag="nident", name="nident")
        ones_col = persist.tile([128, 1], F32, tag="ones_col", name="ones_col")
        lam_b = persist.tile([128, 1], F32, tag="lam_b", name="lam_b")
        alpha_b = persist.tile([128, 1], F32, tag="alpha_b", name="alpha_b")
        nalpha_b = persist.tile([128, 1], F32, tag="nalpha_b", name="nalpha_b")
        beta_b = persist.tile([128, 1], F32, tag="beta_b", name="beta_b")
        dacc = persist.tile([128, 2], F32, tag="dacc", name="dacc")
        sc = persist.tile([1, 12], F32, tag="sc", name="sc")
        # sc slots: 0=rTr, 1=inv_rTr, 2=pAp, 3=alpha, 4=rTrNew, 5=beta,
        # 6,7=tmp, 8=nalpha
        scr = persist.tile([128, 960], F32, tag="scr", name="scr")
        scr2 = persist.tile([128, 960], F32, tag="scr2", name="scr2")
        jnk = persist.tile([128, 960], F32, tag="jnk", name="jnk")
        lp16_re = persist.tile([128, 960], F16, tag="lp16_re", name="lp16_re")
        lp16_im = persist.tile([128, 960], F16, tag="lp16_im", name="lp16_im")

        def ap_tiles():
            return (aps.tile([128, 480], F32, tag="apre0", name="apre0"),
                    aps.tile([128, 480], F32, tag="apre1", name="apre1"),
                    aps.tile([128, 480], F32, tag="apim0", name="apim0"),
                    aps.tile([128, 480], F32, tag="apim1", name="apim1"))

        def emit_full(zr16, zi16, chain, consume):
            """Full-width side with packed-K tail. psum(re,im) per m-block.

            zr16's partitions 64:127 @ cols 640:960 must hold zi16's block 2
            (the dup DMA every producer does)."""
            if chain == "F":
                tB, tC = fi, nfi
                pkre, pkim = pk_f_re, pk_f_im
            else:
                tB, tC = nfi, fi
                pkre, pkim = pk_g_re, pk_g_im
            for mt, (ms, msz) in enumerate(KT):
                P = ps.tile([128, 322], F32, tag="P", name="P")
                Q = ps.tile([128, 322], F32, tag="Q", name="Q")
                pre = P[0:msz, 0:320]
                pim = Q[0:msz, 0:320]
                zr0 = zr16[0:128, 128 * mt:128 * mt + msz]
                zr1 = zr16[0:128, 320 + 128 * mt:320 + 128 * mt + msz]
                zpk = zr16[0:128, 640 + 128 * mt:640 + 128 * mt + msz]
                zi0 = zi16[0:128, 128 * mt:128 * mt + msz]
                zi1 = zi16[0:128, 320 + 128 * mt:320 + 128 * mt + msz]
                mm = nc.tensor.matmul
                mm(pre, zr0, fr[:, _mblk(0)], start=True, stop=False)
                mm(pim, zr0, tB[:, _mblk(0)], start=True, stop=False)
                mm(pre, zr1, fr[:, _mblk(1)], start=False, stop=False)
                mm(pim, zr1, tB[:, _mblk(1)], start=False, stop=False)
                mm(pre, zpk, pkre[:, 0:320], start=False, stop=False)
                mm(pim, zpk, pkim[:, 0:320], start=False, stop=False)
                mm(pre, zi0, tC[:, _mblk(0)], start=False, stop=False)
                mm(pim, zi0, fr[:, _mblk(0)], start=False, stop=False)
                mm(pre, zi1, tC[:, _mblk(1)], start=False, stop=True)
                mm(pim, zi1, fr[:, _mblk(1)], start=False, stop=True)
                consume(mt, msz, pre, pim)

        def emit_mirror(zr16, zi16, chain, dstr, dsti, eng_lo, eng_hi):
            """Half-spectrum side: products A=zr@FrL, D'=zr@FiL, B'=zi@FiL,
            C'=zi@FrL; +/- assembly fills both halves of dstr/dsti."""
            for mt, (ms, msz) in enumerate(KT):
                P = ps.tile([128, 322], F32, tag="P", name="P")
                Q = ps.tile([128, 322], F32, tag="Q", name="Q")
                mm = nc.tensor.matmul
                for kt, (ks, ksz) in enumerate(KT):
                    zrk = zr16[0:ksz, 320 * kt + 128 * mt:320 * kt + 128 * mt + msz]
                    mm(P[0:msz, 0:322], zrk, m_frfi[0:ksz, 322 * kt:322 * kt + 322],
                       start=(kt == 0), stop=(kt == 2))
                for kt, (ks, ksz) in enumerate(KT):
                    zik = zi16[0:ksz, 320 * kt + 128 * mt:320 * kt + 128 * mt + msz]
                    mm(Q[0:msz, 0:322], zik, m_fifr[0:ksz, 322 * kt:322 * kt + 322],
                       start=(kt == 0), stop=(kt == 2))
                # assembly: 4 DVE TTs reading both psums directly; if the
                # s2s2d2 both-PSUM limit bites, set CG_PSB=1 to copy P first
                Psb = tmp32.tile([128, 322], F32, tag="Psb", name="Psb")
                nc.scalar.copy(Psb[0:msz, :], P[0:msz, :])
                Puse = Psb
                lo_r = dstr[0:msz, 320 * mt:320 * mt + ML]
                hi_r = dstr[0:msz, 320 * mt + ML:320 * mt + 320]
                lo_i = dsti[0:msz, 320 * mt:320 * mt + ML]
                hi_i = dsti[0:msz, 320 * mt + ML:320 * mt + 320]
                sbA = Puse[0:msz, 0:ML]
                sbAr = Puse[0:msz, 159:0:-1]
                sbD = Puse[0:msz, ML:322]
                sbDr = Puse[0:msz, 320:161:-1]
                qB = Q[0:msz, 0:ML]
                qBr = Q[0:msz, 159:0:-1]
                qC = Q[0:msz, ML:322]
                qCr = Q[0:msz, 320:161:-1]
                if chain == "F":
                    eng_lo.tensor_sub(lo_r, sbA, qB)          # A - B'
                    eng_hi.tensor_add(hi_r, sbAr, qBr)        # (A + B')rev
                    eng_lo.tensor_add(lo_i, sbD, qC)          # C' + D'
                    eng_hi.tensor_sub(hi_i, qCr, sbDr)        # (C' - D')rev
                else:
                    eng_lo.tensor_add(lo_r, sbA, qB)
                    eng_hi.tensor_sub(hi_r, sbAr, qBr)
                    eng_lo.tensor_sub(lo_i, qC, sbD)
                    eng_hi.tensor_add(hi_i, qCr, sbDr)
                if mt == 2:
                    nc.sync.dma_start(dstr[64:128, 640:960], dsti[0:64, 640:960])

        def cmul_to_fp16(ar, ai, br, bi, outr, outi):
            """(outr + i outi) = (ar + i ai)(br + i bi); fp16 out + dup."""
            t1 = stg16.tile([128, 960], F16, tag="mm_t1", name="mm_t1")
            t2 = stg16.tile([128, 960], F16, tag="mm_t2", name="mm_t2")
            t3 = stg16.tile([128, 960], F16, tag="mm_t3", name="mm_t3")
            t4 = stg16.tile([128, 960], F16, tag="mm_t4", name="mm_t4")
            nc.gpsimd.tensor_mul(t1[:], ar[:], br[:])
            nc.gpsimd.tensor_mul(t2[:], ai[:], bi[:])
            nc.vector.tensor_mul(t3[:], ar[:], bi[:])
            nc.vector.tensor_mul(t4[:], ai[:], br[:])
            nc.vector.tensor_sub(outr[:], t1[:], t2[:])
            nc.vector.tensor_add(outi[:], t3[:], t4[:])
            nc.sync.dma_start(outr[64:128, 640:960], outi[0:64, 640:960])

        def ifft_and_combine(c, inr16, ini16, first, last, aptiles):
            """S3 (mirror G) + S4 (full G) + conj(s_c) products + PSUM accum."""
            apre0, apre1, apim0, apim1 = aptiles
            s3r = stg16.tile([128, 960], F16, tag="s3r", name="s3r")
            s3i = stg16.tile([128, 960], F16, tag="s3i", name="s3i")
            emit_mirror(inr16, ini16, "G", s3r, s3i, nc.vector, nc.vector)

            u4r = tmp32.tile([128, 960], F32, tag="u4r", name="u4r")
            u4i = tmp32.tile([128, 960], F32, tag="u4i", name="u4i")

            def consume4(mt, msz, pre, pim):
                nc.scalar.copy(u4r[0:msz, _mblk(mt)], pre[0:msz, :])
                nc.scalar.copy(u4i[0:msz, _mblk(mt)], pim[0:msz, :])
            emit_full(s3r, s3i, "G", consume4)
            if _dbg == "stage" and c == 0:
                for t, (s, sz) in enumerate(KT):
                    nc.sync.dma_start(d["dbg16"].ap()[0, s:s + sz, :], s3r[0:sz, _mblk(t)])
                    nc.sync.dma_start(d["dbg16"].ap()[1, s:s + sz, :], s3i[0:sz, _mblk(t)])
                    nc.sync.dma_start(d["dbg32"].ap()[0, s:s + sz, :], u4r[0:sz, _mblk(t)])
                    nc.sync.dma_start(d["dbg32"].ap()[1, s:s + sz, :], u4i[0:sz, _mblk(t)])

            def finish():
                # ap_re += sr*u4r + si*u4i ; ap_im += sr*u4i - si*u4r
                w1 = stg16.tile([128, 960], F16, tag="w1", name="w1")
                w2 = stg16.tile([128, 960], F16, tag="w2", name="w2")
                w3 = stg16.tile([128, 960], F16, tag="w3", name="w3")
                w4 = stg16.tile([128, 960], F16, tag="w4", name="w4")
                nc.vector.tensor_mul(w1[:], sr[c][:], u4r[:])
                nc.gpsimd.tensor_mul(w2[:], si[c][:], u4i[:])
                nc.vector.tensor_mul(w3[:], sr[c][:], u4i[:])
                nc.gpsimd.tensor_mul(w4[:], si[c][:], u4r[:])
                mm = nc.tensor.matmul
                mm(apre0[:, :], ident[:, :], w1[:, 0:480], start=first, stop=False)
                mm(apre1[:, :], ident[:, :], w1[:, 480:960], start=first, stop=False)
                mm(apim0[:, :], ident[:, :], w3[:, 0:480], start=first, stop=False)
                mm(apim1[:, :], ident[:, :], w3[:, 480:960], start=first, stop=False)
                mm(apre0[:, :], ident[:, :], w2[:, 0:480], start=False, stop=last)
                mm(apre1[:, :], ident[:, :], w2[:, 480:960], start=False, stop=last)
                mm(apim0[:, :], nident[:, :], w4[:, 0:480], start=False, stop=last)
                mm(apim1[:, :], nident[:, :], w4[:, 480:960], start=False, stop=last)
            return finish

        def seed_ap(vr, vi, aptiles):
            """Start the PSUM ap accumulation group with lam * (vr, vi)."""
            apre0, apre1, apim0, apim1 = aptiles
            nc.scalar.mul(lp16_re[:], vr[:], lam_b[:, 0:1])
            nc.scalar.mul(lp16_im[:], vi[:], lam_b[:, 0:1])
            mm = nc.tensor.matmul
            mm(apre0[:, :], ident[:, :], lp16_re[:, 0:480], start=True, stop=False)
            mm(apre1[:, :], ident[:, :], lp16_re[:, 480:960], start=True, stop=False)
            mm(apim0[:, :], ident[:, :], lp16_im[:, 0:480], start=True, stop=False)
            mm(apim1[:, :], ident[:, :], lp16_im[:, 480:960], start=True, stop=False)

        def reduce_dacc(slot):
            pd = ps.tile([128, 322], F32, tag="P", name="pdot")
            nc.tensor.matmul(pd[0:1, 0:2], ones_col[:, 0:1], dacc[:, 0:2],
                             start=True, stop=True)
            nc.vector.tensor_copy(sc[0:1, 6:8], pd[0:1, 0:2])
            nc.vector.tensor_add(sc[0:1, slot:slot + 1], sc[0:1, 6:7],
                                 sc[0:1, 7:8])

        def dot_self(a_re, a_im, slot):
            nc.scalar.activation(jnk[:], a_re[:], ACT_SQ, accum_out=dacc[:, 0:1])
            nc.scalar.activation(jnk[:], a_im[:], ACT_SQ, accum_out=dacc[:, 1:2])
            reduce_dacc(slot)

        def dot_p_ap(aptiles, slot):
            apre0, apre1, apim0, apim1 = aptiles
            nc.vector.tensor_mul(scr[:, 0:480], p_re[:, 0:480], apre0[:, :])
            nc.vector.tensor_mul(scr[:, 480:960], p_re[:, 480:960], apre1[:, :])
            nc.vector.tensor_mul(scr2[:, 0:480], p_im[:, 0:480], apim0[:, :])
            nc.vector.tensor_mul(scr2[:, 480:960], p_im[:, 480:960], apim1[:, :])
            nc.scalar.activation(jnk[:], scr[:], ACT_CP, accum_out=dacc[:, 0:1])
            nc.scalar.activation(jnk[:], scr2[:], ACT_CP, accum_out=dacc[:, 1:2])
            reduce_dacc(slot)

        # ---- load constants + inputs ----
        for nm, t in [("c_fr", fr), ("c_fi", fi), ("c_nfi", nfi),
                      ("pk_f_re", pk_f_re), ("pk_f_im", pk_f_im),
                      ("pk_g_re", pk_g_re), ("pk_g_im", pk_g_im),
                      ("m_frfi", m_frfi), ("m_fifr", m_fifr), ("ident", ident),
                      ("nident", nident), ("ones_col", ones_col),
                      ("lam_b", lam_b)]:
            nc.sync.dma_start(t[:], d[nm].ap())
        for c in range(C):
            load_blocks(sr[c], d["s_re"].ap()[c])
            load_blocks(si[c], d["s_im"].ap()[c])
            zero_pad(sr[c])
            zero_pad(si[c])
        load_blocks(mask1, d["mask"].ap())
        zero_pad(mask1)
        load_blocks(x_re, d["x_re"].ap())
        load_blocks(x_im, d["x_im"].ap())
        zero_pad(x_re)
        zero_pad(x_im)
        nc.vector.tensor_mul(mask2[:], mask1[:], mask1[:])

        # ---- phase 1: rhs = AH(mask*y) + lam*x ; r0 = p0 = rhs ; x0 = 0 ----
        aptiles = ap_tiles()
        seed_ap(x_re, x_im, aptiles)

        def make_my(c):
            yr = tmp32.tile([128, 960], F32, tag="yr", name="yr")
            yi = tmp32.tile([128, 960], F32, tag="yi", name="yi")
            load_blocks(yr, d["y_re"].ap()[c])
            load_blocks(yi, d["y_im"].ap()[c])
            zero_pad(yr, nc.gpsimd)
            zero_pad(yi, nc.gpsimd)
            myr = stg16.tile([128, 960], F16, tag="spr", name="myr")
            myi = stg16.tile([128, 960], F16, tag="spi", name="myi")
            nc.vector.tensor_mul(myr[:], yr[:], mask1[:])
            nc.vector.tensor_mul(myi[:], yi[:], mask1[:])
            nc.sync.dma_start(myr[64:128, 640:960], myi[0:64, 640:960])
            return myr, myi

        my_next = make_my(0)
        pending = None
        for c in range(C):
            myr, myi = my_next
            if c + 1 < C:
                my_next = make_my(c + 1)
            fin = ifft_and_combine(c, myr, myi, first=False, last=(c == C - 1),
                                   aptiles=aptiles)
            if pending is not None:
                pending()
            pending = fin
        pending()

        # r0 = ap (psum) ; p0 = r0 ; x0 = 0
        apre0, apre1, apim0, apim1 = aptiles
        nc.scalar.copy(r_re[:, 0:480], apre0[:, :])
        nc.scalar.copy(r_re[:, 480:960], apre1[:, :])
        nc.scalar.copy(r_im[:, 0:480], apim0[:, :])
        nc.scalar.copy(r_im[:, 480:960], apim1[:, :])
        nc.scalar.copy(p_re[:], r_re[:])
        nc.scalar.copy(p_im[:], r_im[:])
        nc.vector.memset(x_re[:], 0.0)
        nc.vector.memset(x_im[:], 0.0)

        dot_self(r_re, r_im, 0)          # rTr0
        nc.vector.reciprocal(sc[0:1, 1:2], sc[0:1, 0:1])

        if _dbg in ("rhs", "stage"):
            nc.scalar.copy(x_re[:], r_re[:])
            nc.scalar.copy(x_im[:], r_im[:])

        # ---- phase 2: CG iterations ----
        def cg_iteration(trim):
            aptiles = ap_tiles()
            spr0 = stg16.tile([128, 960], F16, tag="spr", name="spr")
            spi0 = stg16.tile([128, 960], F16, tag="spi", name="spi")
            cmul_to_fp16(sr[0], si[0], p_re, p_im, spr0, spi0)
            seed_ap(p_re, p_im, aptiles)
            sp_next = (spr0, spi0)
            pending = None
            for c in range(C):
                spr, spi = sp_next
                s1r = stg16.tile([128, 960], F16, tag="s1r", name="s1r")
                s1i = stg16.tile([128, 960], F16, tag="s1i", name="s1i")
                emit_mirror(spr, spi, "F", s1r, s1i, nc.vector, nc.vector)

                wr = stg16.tile([128, 960], F16, tag="wr", name="wr")
                wi = stg16.tile([128, 960], F16, tag="wi", name="wi")

                def consume2(mt, msz, pre, pim):
                    nc.vector.tensor_mul(wr[0:msz, _mblk(mt)], pre[0:msz, :],
                                         mask2[0:msz, _mblk(mt)])
                    nc.vector.tensor_mul(wi[0:msz, _mblk(mt)], pim[0:msz, :],
                                         mask2[0:msz, _mblk(mt)])
                    if mt == 2:
                        nc.sync.dma_start(wr[64:128, 640:960],
                                          wi[0:64, 640:960])
                emit_full(s1r, s1i, "F", consume2)

                if c + 1 < C:
                    sp_next = stg16.tile([128, 960], F16, tag="spr", name="spr"), \
                        stg16.tile([128, 960], F16, tag="spi", name="spi")
                    cmul_to_fp16(sr[c + 1], si[c + 1], p_re, p_im,
                                 sp_next[0], sp_next[1])
                fin = ifft_and_combine(c, wr, wi, first=False, last=(c == C - 1),
                                       aptiles=aptiles)
                if pending is not None:
                    pending()
                pending = fin

            pending()
            dot_p_ap(aptiles, 2)                                        # pAp
            nc.vector.reciprocal(sc[0:1, 6:7], sc[0:1, 2:3])
            nc.vector.tensor_mul(sc[0:1, 3:4], sc[0:1, 0:1], sc[0:1, 6:7])
            nc.vector.tensor_scalar_mul(sc[0:1, 8:9], sc[0:1, 3:4], -1.0)
            nc.gpsimd.partition_broadcast(alpha_b[:, 0:1], sc[0:1, 3:4])
            apre0, apre1, apim0, apim1 = aptiles

            # x += alpha p
            nc.vector.affine_then_add(x_re[:], p_re[:], x_re[:],
                                      scale=alpha_b[:, 0:1], bias=0.0)
            nc.vector.affine_then_add(x_im[:], p_im[:], x_im[:],
                                      scale=alpha_b[:, 0:1], bias=0.0)
            if trim:
                return
            nc.gpsimd.partition_broadcast(nalpha_b[:, 0:1], sc[0:1, 8:9])
            # r -= alpha Ap
            nc.vector.affine_then_add(r_re[:, 0:480], apre0[:, :],
                                      r_re[:, 0:480], scale=nalpha_b[:, 0:1],
                                      bias=0.0)
            nc.vector.affine_then_add(r_re[:, 480:960], apre1[:, :],
                                      r_re[:, 480:960], scale=nalpha_b[:, 0:1],
                                      bias=0.0)
            nc.vector.affine_then_add(r_im[:, 0:480], apim0[:, :],
                                      r_im[:, 0:480], scale=nalpha_b[:, 0:1],
                                      bias=0.0)
            nc.vector.affine_then_add(r_im[:, 480:960], apim1[:, :],
                                      r_im[:, 480:960], scale=nalpha_b[:, 0:1],
                                      bias=0.0)

            dot_self(r_re, r_im, 4)                                     # rTrNew
            nc.vector.tensor_mul(sc[0:1, 5:6], sc[0:1, 4:5], sc[0:1, 1:2])
            nc.vector.tensor_copy(sc[0:1, 0:1], sc[0:1, 4:5])
            nc.vector.reciprocal(sc[0:1, 1:2], sc[0:1, 4:5])
            nc.gpsimd.partition_broadcast(beta_b[:, 0:1], sc[0:1, 5:6])
            # p = beta p + r
            nc.vector.affine_then_add(p_re[:], p_re[:], r_re[:],
                                      scale=beta_b[:, 0:1], bias=0.0)
            nc.vector.affine_then_add(p_im[:], p_im[:], r_im[:],
                                      scale=beta_b[:, 0:1], bias=0.0)

        if _dbg not in ("rhs", "stage"):
            if N_ITER > 1:
                with tc.For_i(0, N_ITER - 1, 1):
                    cg_iteration(trim=False)
            cg_iteration(trim=True)

        for t, (s, sz) in enumerate(KT):
            nc.sync.dma_start(d["out"].ap()[0, s:s + sz, :], x_re[0:sz, _mblk(t)])
            nc.sync.dma_start(d["out"].ap()[1, s:s + sz, :], x_im[0:sz, _mblk(t)])

    nc.compile()
    return nc


def kernel(lambdaa, x_re, x_im, y_re, y_im, smaps_re, smaps_im, mask):
    global _PROGRAM
    lambdaa = np.asarray(lambdaa, np.float32)
    arrs = {
        "x_re": x_re, "x_im": x_im, "y_re": y_re, "y_im": y_im,
    }
    arrs = {k: np.ascontiguousarray(np.asarray(v, np.float32))
            for k, v in arrs.items()}
    arrs["s_re"] = np.ascontiguousarray(np.asarray(smaps_re, np.float16))
    arrs["s_im"] = np.ascontiguousarray(np.asarray(smaps_im, np.float16))
    mask = np.ascontiguousarray(np.asarray(mask, np.float32))

    _selfcheck()
    if _PROGRAM is None:
        _PROGRAM = _build_program()
    nc = _PROGRAM

    consts = _build_consts()
    lam_b = np.full((128, 1), float(lambdaa[0]), np.float32)
    in_maps = []
    for i in range(B):
        in_maps.append({
            **{k: v[i] for k, v in arrs.items()},
            "mask": np.ascontiguousarray(mask[i, 0]),
            "lam_b": lam_b,
            **consts,
        })

    res = bass_utils.run_bass_kernel_spmd(nc, in_maps, core_ids=list(range(B)),
                                          trace=TRACE)
    kernel._last_result = res
    out = np.empty((B, H, H, 2), np.float32)
    for i in range(B):
        o = res.results[i]["out"]
        out[i, :, :, 0] = o[0]
        out[i, :, :, 1] = o[1]
    return out


# revision 23
# speedup vs baseline: 1.2735x; 1.2302x over previous
"""CG-SENSE MRI reconstruction (nn_CGClass) on 8 Trainium2 NeuronCores.

Strategy: data-parallel over batch (B=8 -> 1 sample per core). Per core the
whole CG (10 iterations) runs on-chip. fft2/ifft2 are dense 320-point DFTs
done on the tensor engine as fp16 matmuls via the transpose-free primitive
OUT = Z^T @ A (data stationary, DFT matrix moving); applying it twice gives
F @ Z @ F with no transposes. CG state stays fp32; only matmul operands are
fp16 (measured end-to-end error ~3e-4, reference tol is far looser).

Layout: each 320x320 real array lives in SBUF as [128, 960]: free-dim block
t in {0,1,2} holds image rows [128t : 128t+{128,128,64}]. Block 2 uses
partitions 0..63; pad regions are kept zero (NaN hygiene for reductions).

The ragged K=64 block-2 contraction tail is packed: each imag tile's block 2
is DMA'd into the matching real tile's partitions 64:127, so one K=128
matmul with a host-stacked [table_for_re; table_for_im] moving operand
covers both 64-row tails -> 10 matmuls per m-block instead of 12.
"""
import os
from contextlib import ExitStack

import numpy as np

import concourse.bass as bass  # noqa: F401
import concourse.tile as tile
from concourse import mybir, bass_utils, bacc

F32 = mybir.dt.float32
F16 = mybir.dt.float16
MULT = mybir.AluOpType.mult
ADD = mybir.AluOpType.add

H = 320
B, C = 8, 12
N_ITER = int(os.environ.get("CG_ITERS", "10"))
KT = [(0, 128), (128, 128), (256, 64)]  # (row_start, rows) per block

_PROGRAM = None
TRACE = bool(os.environ.get("CG_TRACE"))
UNROLL = bool(os.environ.get("CG_UNROLL"))
PACK = os.environ.get("CG_PACK", "1") != "0"


def _mblk(t):
    return slice(320 * t, 320 * t + 320)


def _build_consts():
    j = np.arange(H)
    ang = -2.0 * np.pi * np.outer(j, j) / H
    scale = 1.0 / np.sqrt(H)
    Fr = (np.cos(ang) * scale).astype(np.float32)
    Fi = (np.sin(ang) * scale).astype(np.float32)

    def blocks(a):
        out = np.zeros((128, 960), np.float32)
        for t, (s, sz) in enumerate(KT):
            out[:sz, 320 * t:320 * t + 320] = a[s:s + sz]
        # block-2 rows duplicated at partitions 64..127 for row-group-packed
        # K=64 matmuls (stationary pairs at array rows 0-63 / 64-127)
        out[64:128, 640:960] = out[0:64, 640:960]
        return out

    def pk(ta, tb):
        out = np.zeros((128, 320), np.float32)
        out[0:64] = ta[256:320]
        out[64:128] = tb[256:320]
        return out

    return {
        "c_fr": blocks(Fr).astype(np.float16),
        "c_fi": blocks(Fi).astype(np.float16),
        "c_nfi": blocks(-Fi).astype(np.float16),
        # packed block-2 moving tables: [table_for_zr rows 256:320 ;
        # table_for_zi rows 256:320] per chain / component
        "pk_f_re": pk(Fr, -Fi).astype(np.float16),
        "pk_f_im": pk(Fi, Fr).astype(np.float16),
        "pk_g_re": pk(Fr, Fi).astype(np.float16),
        "pk_g_im": pk(-Fi, Fr).astype(np.float16),
        "ones_col": np.ones((128, 1), np.float32),
        "ones_row": np.ones((1, 128), np.float32),
    }


def _build_program():
    nc = bacc.Bacc("TRN2", target_bir_lowering=False, debug=False)

    d = {}
    d["x_re"] = nc.dram_tensor("x_re", [H, H], F32, kind="ExternalInput")
    d["x_im"] = nc.dram_tensor("x_im", [H, H], F32, kind="ExternalInput")
    d["y_re"] = nc.dram_tensor("y_re", [C, H, H], F32, kind="ExternalInput")
    d["y_im"] = nc.dram_tensor("y_im", [C, H, H], F32, kind="ExternalInput")
    d["s_re"] = nc.dram_tensor("s_re", [C, H, H], F16, kind="ExternalInput")
    d["s_im"] = nc.dram_tensor("s_im", [C, H, H], F16, kind="ExternalInput")
    d["mask"] = nc.dram_tensor("mask", [H, H], F32, kind="ExternalInput")
    d["lam_b"] = nc.dram_tensor("lam_b", [128, 1], F32, kind="ExternalInput")
    d["c_fr"] = nc.dram_tensor("c_fr", [128, 960], F16, kind="ExternalInput")
    d["c_fi"] = nc.dram_tensor("c_fi", [128, 960], F16, kind="ExternalInput")
    d["c_nfi"] = nc.dram_tensor("c_nfi", [128, 960], F16, kind="ExternalInput")
    d["pk_f_re"] = nc.dram_tensor("pk_f_re", [128, 320], F16, kind="ExternalInput")
    d["pk_f_im"] = nc.dram_tensor("pk_f_im", [128, 320], F16, kind="ExternalInput")
    d["pk_g_re"] = nc.dram_tensor("pk_g_re", [128, 320], F16, kind="ExternalInput")
    d["pk_g_im"] = nc.dram_tensor("pk_g_im", [128, 320], F16, kind="ExternalInput")
    d["ones_col"] = nc.dram_tensor("ones_col", [128, 1], F32, kind="ExternalInput")
    d["ones_row"] = nc.dram_tensor("ones_row", [1, 128], F32, kind="ExternalInput")
    d["out"] = nc.dram_tensor("out", [2, H, H], F32, kind="ExternalOutput")

    with tile.TileContext(nc) as tc, ExitStack() as ctx:
        persist = ctx.enter_context(tc.tile_pool(name="persist", bufs=1))
        stg16 = ctx.enter_context(tc.tile_pool(name="stg16", bufs=2))
        tmp32 = ctx.enter_context(tc.tile_pool(name="tmp32", bufs=2))
        ps = ctx.enter_context(tc.tile_pool(name="ps", bufs=3, space="PSUM"))
        pss = ctx.enter_context(tc.tile_pool(name="pss", bufs=1, space="PSUM"))

        def load_blocks(dst, src_ap):
            for t, (s, sz) in enumerate(KT):
                nc.sync.dma_start(dst[0:sz, _mblk(t)], src_ap[s:s + sz, :])

        def zero_pad(t32, eng=None):
            (eng or nc.vector).memset(t32[64:128, 640:960], 0.0)

        # ---- persistent tiles ----
        sr = [persist.tile([128, 960], F16, tag=f"sr{c}", name=f"sr{c}") for c in range(C)]
        si = [persist.tile([128, 960], F16, tag=f"si{c}", name=f"si{c}") for c in range(C)]
        mask2 = persist.tile([128, 960], F32, tag="mask2", name="mask2")
        mask1 = persist.tile([128, 960], F32, tag="mask1", name="mask1")
        p_re = persist.tile([128, 960], F32, tag="p_re", name="p_re")
        p_im = persist.tile([128, 960], F32, tag="p_im", name="p_im")
        r_re = persist.tile([128, 960], F32, tag="r_re", name="r_re")
        r_im = persist.tile([128, 960], F32, tag="r_im", name="r_im")
        x_re = persist.tile([128, 960], F32, tag="x_re", name="x_re")
        x_im = persist.tile([128, 960], F32, tag="x_im", name="x_im")
        ap_re = persist.tile([128, 960], F32, tag="ap_re", name="ap_re")
        ap_im = persist.tile([128, 960], F32, tag="ap_im", name="ap_im")
        fr = persist.tile([128, 960], F16, tag="fr", name="fr")
        fi = persist.tile([128, 960], F16, tag="fi", name="fi")
        nfi = persist.tile([128, 960], F16, tag="nfi", name="nfi")
        pk_f_re = persist.tile([128, 320], F16, tag="pk_f_re", name="pk_f_re")
        pk_f_im = persist.tile([128, 320], F16, tag="pk_f_im", name="pk_f_im")
        pk_g_re = persist.tile([128, 320], F16, tag="pk_g_re", name="pk_g_re")
        pk_g_im = persist.tile([128, 320], F16, tag="pk_g_im", name="pk_g_im")
        ones_col = persist.tile([128, 1], F32, tag="ones_col", name="ones_col")
        ones_row = persist.tile([1, 128], F32, tag="ones_row", name="ones_row")
        lam_b = persist.tile([128, 1], F32, tag="lam_b", name="lam_b")
        alpha_b = persist.tile([128, 1], F32, tag="alpha_b", name="alpha_b")
        beta_b = persist.tile([128, 1], F32, tag="beta_b", name="beta_b")
        dacc = persist.tile([128, 2], F32, tag="dacc", name="dacc")
        sc = persist.tile([1, 8], F32, tag="sc", name="sc")
        # sc slots: 0=rTr, 1=inv_rTr, 2=pAp, 3=alpha, 4=rTrNew, 5=beta, 6=tmp
        scr = persist.tile([128, 960], F32, tag="scr", name="scr")
        scr2 = persist.tile([128, 960], F32, tag="scr2", name="scr2")
        jnk = persist.tile([128, 960], F32, tag="jnk", name="jnk")

        def emit_side(zr16, zi16, chain, consume):
            """psum(re,im) per m-block of Z^T @ A, complex. consume(mt,msz,pre,pim).

            With PACK, zr16's partitions 64:127 @ cols 640:960 must hold
            zi16's block 2 (producers dup via DMA); the block-2 tails of the
            zr and zi contractions then merge into one K=128 matmul against
            the host-stacked pk tables."""
            if chain == "F":
                tB, tC = fi, nfi
                pkre, pkim = pk_f_re, pk_f_im
            else:  # G = conj(F)
                tB, tC = nfi, fi
                pkre, pkim = pk_g_re, pk_g_im
            mm = nc.tensor.matmul
            for mt, (ms, msz) in enumerate(KT):
                pre = ps.tile([128, 320], F32, tag="ps_re", name="ps_re", bufs=4)
                pim = ps.tile([128, 320], F32, tag="ps_im", name="ps_im")
                prs = pre[0:msz, :]
                pis = pim[0:msz, :]
                if PACK:
                    zr0 = zr16[0:128, 128 * mt:128 * mt + msz]
                    zr1 = zr16[0:128, 320 + 128 * mt:320 + 128 * mt + msz]
                    zpk = zr16[0:128, 640 + 128 * mt:640 + 128 * mt + msz]
                    zi0 = zi16[0:128, 128 * mt:128 * mt + msz]
                    zi1 = zi16[0:128, 320 + 128 * mt:320 + 128 * mt + msz]
                    mm(prs, zr0, fr[:, _mblk(0)], start=True, stop=False)
                    mm(pis, zr0, tB[:, _mblk(0)], start=True, stop=False)
                    mm(prs, zr1, fr[:, _mblk(1)], start=False, stop=False)
                    mm(pis, zr1, tB[:, _mblk(1)], start=False, stop=False)
                    mm(prs, zpk, pkre[:, 0:320], start=False, stop=False)
                    mm(pis, zpk, pkim[:, 0:320], start=False, stop=False)
                    mm(prs, zi0, tC[:, _mblk(0)], start=False, stop=False)
                    mm(pis, zi0, fr[:, _mblk(0)], start=False, stop=False)
                    mm(prs, zi1, tC[:, _mblk(1)], start=False, stop=True)
                    mm(pis, zi1, fr[:, _mblk(1)], start=False, stop=True)
                else:
                    if chain == "F":
                        mov = [(zr16, fr, "re"), (zr16, fi, "im"),
                               (zi16, nfi, "re"), (zi16, fr, "im")]
                    else:
                        mov = [(zr16, fr, "re"), (zr16, nfi, "im"),
                               (zi16, fi, "re"), (zi16, fr, "im")]
                    cnt = {"re": 0, "im": 0}
                    for kt, (ks, ksz) in enumerate(KT):
                        for z, a, dst in mov:
                            lo = 320 * kt + 128 * mt
                            pt = (pre if dst == "re" else pim)[0:msz, :]
                            cnt[dst] += 1
                            zt = z[0:ksz, lo:lo + msz]
                            at = a[0:ksz, _mblk(kt)]
                            mm(pt, zt, at, start=(cnt[dst] == 1),
                               stop=(cnt[dst] == 6))
                consume(mt, msz, pre, pim)

        def cmul_to_fp16(ar, ai, br, bi, outr, outi):
            """(outr + i outi) = (ar + i ai)(br + i bi); fp32 in, fp16 out."""
            t1 = tmp32.tile([128, 960], F32, tag="mm_t1", name="mm_t1")
            t2 = tmp32.tile([128, 960], F32, tag="mm_t2", name="mm_t2")
            t3 = tmp32.tile([128, 960], F32, tag="mm_t3", name="mm_t3")
            t4 = tmp32.tile([128, 960], F32, tag="mm_t4", name="mm_t4")
            nc.gpsimd.tensor_mul(t1[:], ar[:], br[:])
            nc.gpsimd.tensor_mul(t2[:], ai[:], bi[:])
            nc.gpsimd.tensor_mul(t3[:], ar[:], bi[:])
            nc.gpsimd.tensor_mul(t4[:], ai[:], br[:])
            nc.vector.tensor_sub(outr[:], t1[:], t2[:])
            nc.vector.tensor_add(outi[:], t3[:], t4[:])
            nc.sync.dma_start(outr[64:128, 640:960], outi[0:64, 640:960])

        def combine_coil(c, u4r, u4i):
            """ap += conj(s_c) * u4 (fp32)."""
            t1 = tmp32.tile([128, 960], F32, tag="mm_t1", name="mm_t1")
            t2 = tmp32.tile([128, 960], F32, tag="mm_t2", name="mm_t2")
            t3 = tmp32.tile([128, 960], F32, tag="mm_t3", name="mm_t3")
            t4 = tmp32.tile([128, 960], F32, tag="mm_t4", name="mm_t4")
            nc.vector.tensor_mul(t1[:], sr[c][:], u4r[:])
            nc.vector.tensor_mul(t2[:], si[c][:], u4i[:])
            nc.vector.tensor_mul(t3[:], sr[c][:], u4i[:])
            nc.vector.tensor_mul(t4[:], si[c][:], u4r[:])
            nc.vector.tensor_add(ap_re[:], ap_re[:], t1[:])
            nc.vector.tensor_add(ap_re[:], ap_re[:], t2[:])
            nc.vector.tensor_add(ap_im[:], ap_im[:], t3[:])
            nc.vector.tensor_sub(ap_im[:], ap_im[:], t4[:])

        def ifft_and_combine(c, inr16, ini16):
            """Emit S3/S4 G-chain; return deferred combine closure."""
            s3r = stg16.tile([128, 960], F16, tag="s3r", name="s3r")
            s3i = stg16.tile([128, 960], F16, tag="s3i", name="s3i")

            def consume3(mt, msz, pre, pim):
                nc.scalar.copy(s3r[0:msz, _mblk(mt)], pre[0:msz, :])
                nc.scalar.copy(s3i[0:msz, _mblk(mt)], pim[0:msz, :])
                if mt == 2:
                    nc.sync.dma_start(s3r[64:128, 640:960], s3i[0:64, 640:960])
            emit_side(inr16, ini16, "G", consume3)

            u4r = tmp32.tile([128, 960], F32, tag="u4r", name="u4r")
            u4i = tmp32.tile([128, 960], F32, tag="u4i", name="u4i")
            zero_pad(u4r, nc.gpsimd)
            zero_pad(u4i, nc.gpsimd)

            def consume4(mt, msz, pre, pim):
                nc.scalar.copy(u4r[0:msz, _mblk(mt)], pre[0:msz, :])
                nc.scalar.copy(u4i[0:msz, _mblk(mt)], pim[0:msz, :])
            emit_side(s3r, s3i, "G", consume4)
            return lambda: combine_coil(c, u4r, u4i)

        def dot_to_sc(a_re, b_re, a_im, b_im, slot):
            """sc[0, slot] = sum(a_re*b_re + a_im*b_im) over valid region.

            tensor_tensor_reduce faults on this hardware path, so: self-dots
            use ACT Square+accum_out; cross-dots DVE-mult + ACT Copy+accum.
            """
            SQ = mybir.ActivationFunctionType.Square
            CP = mybir.ActivationFunctionType.Copy
            if a_re is b_re and a_im is b_im:
                nc.scalar.activation(jnk[:], a_re[:], SQ, accum_out=dacc[:, 0:1])
                nc.scalar.activation(jnk[:], a_im[:], SQ, accum_out=dacc[:, 1:2])
            else:
                nc.vector.tensor_mul(scr[:], a_re[:], b_re[:])
                nc.vector.tensor_mul(scr2[:], a_im[:], b_im[:])
                nc.scalar.activation(jnk[:], scr[:], CP, accum_out=dacc[:, 0:1])
                nc.scalar.activation(jnk[:], scr2[:], CP, accum_out=dacc[:, 1:2])
            pd = pss.tile([1, 2], F32, tag="pdot", name="pdot")
            nc.tensor.matmul(pd[0:1, 0:2], ones_col[:, 0:1], dacc[:, 0:2],
                             start=True, stop=True)
            nc.vector.tensor_copy(sc[0:1, 6:8], pd[0:1, 0:2])
            nc.vector.tensor_add(sc[0:1, slot:slot + 1], sc[0:1, 6:7],
                                 sc[0:1, 7:8])

        # ---- load constants + inputs ----
        nc.sync.dma_start(fr[:], d["c_fr"].ap())
        nc.sync.dma_start(fi[:], d["c_fi"].ap())
        nc.sync.dma_start(nfi[:], d["c_nfi"].ap())
        nc.sync.dma_start(pk_f_re[:], d["pk_f_re"].ap())
        nc.sync.dma_start(pk_f_im[:], d["pk_f_im"].ap())
        nc.sync.dma_start(pk_g_re[:], d["pk_g_re"].ap())
        nc.sync.dma_start(pk_g_im[:], d["pk_g_im"].ap())
        nc.sync.dma_start(ones_col[:], d["ones_col"].ap())
        nc.sync.dma_start(ones_row[:], d["ones_row"].ap())
        nc.sync.dma_start(lam_b[:], d["lam_b"].ap())
        for c in range(C):
            load_blocks(sr[c], d["s_re"].ap()[c])
            load_blocks(si[c], d["s_im"].ap()[c])
            zero_pad(sr[c])
            zero_pad(si[c])
        load_blocks(mask1, d["mask"].ap())
        zero_pad(mask1)
        load_blocks(x_re, d["x_re"].ap())
        load_blocks(x_im, d["x_im"].ap())
        zero_pad(x_re)
        zero_pad(x_im)
        nc.vector.tensor_mul(mask2[:], mask1[:], mask1[:])

        nc.vector.memset(ap_re[:], 0.0)
        nc.vector.memset(ap_im[:], 0.0)

        # ---- phase 1: rhs ----
        def make_my(c):
            yr = tmp32.tile([128, 960], F32, tag="yr", name="yr")
            yi = tmp32.tile([128, 960], F32, tag="yi", name="yi")
            load_blocks(yr, d["y_re"].ap()[c])
            load_blocks(yi, d["y_im"].ap()[c])
            zero_pad(yr, nc.gpsimd)
            zero_pad(yi, nc.gpsimd)
            myr = stg16.tile([128, 960], F16, tag="spr", name="myr")
            myi = stg16.tile([128, 960], F16, tag="spi", name="myi")
            nc.vector.tensor_mul(myr[:], yr[:], mask1[:])
            nc.vector.tensor_mul(myi[:], yi[:], mask1[:])
            nc.sync.dma_start(myr[64:128, 640:960], myi[0:64, 640:960])
            return myr, myi

        my_next = make_my(0)
        pending = None
        for c in range(C):
            myr, myi = my_next
            if c + 1 < C:
                my_next = make_my(c + 1)
            comb = ifft_and_combine(c, myr, myi)
            if pending is not None:
                pending()
            pending = comb
        if pending is not None:
            pending()

        # r0 = p0 = rhs = ap + lam*x ; x0 = 0
        nc.vector.tensor_scalar_mul(scr[:], x_re[:], lam_b[:, 0:1])
        nc.vector.tensor_add(r_re[:], ap_re[:], scr[:])
        nc.vector.tensor_scalar_mul(scr2[:], x_im[:], lam_b[:, 0:1])
        nc.vector.tensor_add(r_im[:], ap_im[:], scr2[:])
        nc.scalar.copy(p_re[:], r_re[:])
        nc.scalar.copy(p_im[:], r_im[:])
        nc.vector.memset(x_re[:], 0.0)
        nc.vector.memset(x_im[:], 0.0)

        dot_to_sc(r_re, r_re, r_im, r_im, 0)          # rTr0
        nc.vector.reciprocal(sc[0:1, 1:2], sc[0:1, 0:1])

        # ---- phase 2: CG iterations ----
        def cg_iteration():
            nc.vector.tensor_scalar_mul(ap_re[:], p_re[:], lam_b[:, 0:1])
            nc.vector.tensor_scalar_mul(ap_im[:], p_im[:], lam_b[:, 0:1])

            def make_sp(c):
                spr = stg16.tile([128, 960], F16, tag="spr", name="spr")
                spi = stg16.tile([128, 960], F16, tag="spi", name="spi")
                cmul_to_fp16(sr[c], si[c], p_re, p_im, spr, spi)
                return spr, spi

            sp_next = make_sp(0)
            pending = None
            for c in range(C):
                spr, spi = sp_next
                s1r = stg16.tile([128, 960], F16, tag="s1r", name="s1r")
                s1i = stg16.tile([128, 960], F16, tag="s1i", name="s1i")

                def consume1(mt, msz, pre, pim):
                    nc.scalar.copy(s1r[0:msz, _mblk(mt)], pre[0:msz, :])
                    nc.scalar.copy(s1i[0:msz, _mblk(mt)], pim[0:msz, :])
                    if mt == 2:
                        nc.sync.dma_start(s1r[64:128, 640:960],
                                          s1i[0:64, 640:960])
                emit_side(spr, spi, "F", consume1)

                wr = stg16.tile([128, 960], F16, tag="wr", name="wr")
                wi = stg16.tile([128, 960], F16, tag="wi", name="wi")

                def consume2(mt, msz, pre, pim):
                    nc.vector.tensor_mul(wr[0:msz, _mblk(mt)], pre[0:msz, :],
                                         mask2[0:msz, _mblk(mt)])
                    nc.vector.tensor_mul(wi[0:msz, _mblk(mt)], pim[0:msz, :],
                                         mask2[0:msz, _mblk(mt)])
                    if mt == 2:
                        nc.sync.dma_start(wr[64:128, 640:960],
                                          wi[0:64, 640:960])
                emit_side(s1r, s1i, "F", consume2)
                if pending is not None:
                    pending()

                # prepare next coil's SP before this coil's ifft+combine so the
                # DVE/GpSimd streams feed the PE ahead of the combine chain
                if c + 1 < C:
                    sp_next = make_sp(c + 1)
                pending = ifft_and_combine(c, wr, wi)
            pending()

            dot_to_sc(p_re, ap_re, p_im, ap_im, 2)    # pAp
            nc.vector.reciprocal(sc[0:1, 6:7], sc[0:1, 2:3])
            nc.vector.tensor_mul(sc[0:1, 3:4], sc[0:1, 0:1], sc[0:1, 6:7])  # alpha
            pb = pss.tile([128, 1], F32, tag="pdot", name="pbc")
            nc.tensor.matmul(pb[:, 0:1], ones_row[0:1, :], sc[0:1, 3:4],
                             start=True, stop=True)
            nc.scalar.copy(alpha_b[:, 0:1], pb[:, 0:1])

            nc.vector.tensor_scalar_mul(scr[:], ap_re[:], alpha_b[:, 0:1])
            nc.vector.tensor_sub(r_re[:], r_re[:], scr[:])
            nc.vector.tensor_scalar_mul(scr2[:], ap_im[:], alpha_b[:, 0:1])
            nc.vector.tensor_sub(r_im[:], r_im[:], scr2[:])

            dot_to_sc(r_re, r_re, r_im, r_im, 4)      # rTrNew
            nc.vector.tensor_mul(sc[0:1, 5:6], sc[0:1, 4:5], sc[0:1, 1:2])  # beta
            nc.vector.tensor_copy(sc[0:1, 0:1], sc[0:1, 4:5])
            nc.vector.reciprocal(sc[0:1, 1:2], sc[0:1, 4:5])
            pb2 = pss.tile([128, 1], F32, tag="pdot", name="pbc2")
            nc.tensor.matmul(pb2[:, 0:1], ones_row[0:1, :], sc[0:1, 5:6],
                             start=True, stop=True)
            nc.scalar.copy(beta_b[:, 0:1], pb2[:, 0:1])

            nc.scalar.mul(scr[:], p_re[:], beta_b[:, 0:1])
            nc.scalar.mul(scr2[:], p_im[:], beta_b[:, 0:1])
            nc.vector.tensor_scalar(jnk[:], p_re[:], alpha_b[:, 0:1], None, MULT)
            nc.vector.tensor_add(x_re[:], x_re[:], jnk[:])
            nc.vector.tensor_add(p_re[:], r_re[:], scr[:])
            nc.vector.tensor_scalar(scr[:], p_im[:], alpha_b[:, 0:1], None, MULT)
            nc.vector.tensor_add(p_im[:], r_im[:], scr2[:])
            nc.vector.tensor_add(x_im[:], x_im[:], scr[:])

        if UNROLL:
            for _ in range(N_ITER):
                cg_iteration()
        else:
            with tc.For_i(0, N_ITER, 1):
                cg_iteration()

        for t, (s, sz) in enumerate(KT):
            nc.sync.dma_start(d["out"].ap()[0, s:s + sz, :], x_re[0:sz, _mblk(t)])
            nc.sync.dma_start(d["out"].ap()[1, s:s + sz, :], x_im[0:sz, _mblk(t)])

    nc.compile()
    return nc


def kernel(lambdaa, x_re, x_im, y_re, y_im, smaps_re, smaps_im, mask):
    global _PROGRAM
    lambdaa = np.asarray(lambdaa, np.float32)
    arrs = {
        "x_re": x_re, "x_im": x_im, "y_re": y_re, "y_im": y_im,
    }
    arrs = {k: np.ascontiguousarray(np.asarray(v, np.float32))
            for k, v in arrs.items()}
    arrs["s_re"] = np.ascontiguousarray(np.asarray(smaps_re, np.float16))
    arrs["s_im"] = np.ascontiguousarray(np.asarray(smaps_im, np.float16))
    mask = np.ascontiguousarray(np.asarray(mask, np.float32))

    if _PROGRAM is None:
        _PROGRAM = _build_program()
    nc = _PROGRAM

    consts = _build_consts()
    lam_b = np.full((128, 1), float(lambdaa[0]), np.float32)
    in_maps = []
    for i in range(B):
        in_maps.append({
            **{k: v[i] for k, v in arrs.items()},
            "mask": np.ascontiguousarray(mask[i, 0]),
            "lam_b": lam_b,
            **consts,
        })

    res = bass_utils.run_bass_kernel_spmd(nc, in_maps, core_ids=list(range(B)),
                                          trace=TRACE)
    kernel._last_result = res
    out = np.empty((B, H, H, 2), np.float32)
    for i in range(B):
        o = res.results[i]["out"]
        out[i, :, :, 0] = o[0]
        out[i, :, :, 1] = o[1]
    return out


# revision 24
# speedup vs baseline: 1.4506x; 1.1390x over previous
"""CG-SENSE MRI reconstruction (nn_CGClass) on 8 Trainium2 NeuronCores.

Strategy: data-parallel over batch (B=8 -> 1 sample per core). Per core the
whole CG (10 iterations) runs on-chip. fft2/ifft2 are dense 320-point DFTs
done on the tensor engine as fp16 matmuls via the transpose-free primitive
OUT = Z^T @ A (data stationary, DFT matrix moving); applying it twice gives
F @ Z @ F with no transposes. CG state stays fp32; only matmul operands are
fp16 (measured end-to-end error ~3e-4, reference tol is far looser).

Layout: each 320x320 real array lives in SBUF as [128, 960]: free-dim block
t in {0,1,2} holds image rows [128t : 128t+{128,128,64}]. Block 2 uses
partitions 0..63; pad regions are kept zero (NaN hygiene for reductions).
"""
import os
from contextlib import ExitStack

import numpy as np

import concourse.bass as bass  # noqa: F401
import concourse.tile as tile
from concourse import mybir, bass_utils, bacc

F32 = mybir.dt.float32
F16 = mybir.dt.float16
MULT = mybir.AluOpType.mult
ADD = mybir.AluOpType.add

H = 320
B, C = 8, 12
N_ITER = int(os.environ.get("CG_ITERS", "10"))
KT = [(0, 128), (128, 128), (256, 64)]  # (row_start, rows) per block

_PROGRAM = None
TRACE = bool(os.environ.get("CG_TRACE"))
UNROLL = bool(os.environ.get("CG_UNROLL"))


def _mblk(t):
    return slice(320 * t, 320 * t + 320)


def _build_consts():
    j = np.arange(H)
    ang = -2.0 * np.pi * np.outer(j, j) / H
    scale = 1.0 / np.sqrt(H)
    Fr = (np.cos(ang) * scale).astype(np.float32)
    Fi = (np.sin(ang) * scale).astype(np.float32)

    def blocks(a):
        out = np.zeros((128, 960), np.float32)
        for t, (s, sz) in enumerate(KT):
            out[:sz, 320 * t:320 * t + 320] = a[s:s + sz]
        # block-2 rows duplicated at partitions 64..127 for row-group-packed
        # K=64 matmuls (stationary pairs at array rows 0-63 / 64-127)
        out[64:128, 640:960] = out[0:64, 640:960]
        return out

    return {
        "c_fr": blocks(Fr).astype(np.float16),
        "c_fi": blocks(Fi).astype(np.float16),
        "c_nfi": blocks(-Fi).astype(np.float16),
        "ones_col": np.ones((128, 1), np.float32),
        "ones_row": np.ones((1, 128), np.float32),
    }


def _build_program():
    nc = bacc.Bacc("TRN2", target_bir_lowering=False, debug=False)

    d = {}
    d["x_re"] = nc.dram_tensor("x_re", [H, H], F32, kind="ExternalInput")
    d["x_im"] = nc.dram_tensor("x_im", [H, H], F32, kind="ExternalInput")
    d["y_re"] = nc.dram_tensor("y_re", [C, H, H], F32, kind="ExternalInput")
    d["y_im"] = nc.dram_tensor("y_im", [C, H, H], F32, kind="ExternalInput")
    d["s_re"] = nc.dram_tensor("s_re", [C, H, H], F16, kind="ExternalInput")
    d["s_im"] = nc.dram_tensor("s_im", [C, H, H], F16, kind="ExternalInput")
    d["mask"] = nc.dram_tensor("mask", [H, H], F32, kind="ExternalInput")
    d["lam_b"] = nc.dram_tensor("lam_b", [128, 1], F32, kind="ExternalInput")
    d["c_fr"] = nc.dram_tensor("c_fr", [128, 960], F16, kind="ExternalInput")
    d["c_fi"] = nc.dram_tensor("c_fi", [128, 960], F16, kind="ExternalInput")
    d["c_nfi"] = nc.dram_tensor("c_nfi", [128, 960], F16, kind="ExternalInput")
    d["ones_col"] = nc.dram_tensor("ones_col", [128, 1], F32, kind="ExternalInput")
    d["ones_row"] = nc.dram_tensor("ones_row", [1, 128], F32, kind="ExternalInput")
    d["out"] = nc.dram_tensor("out", [2, H, H], F32, kind="ExternalOutput")

    with tile.TileContext(nc) as tc, ExitStack() as ctx:
        persist = ctx.enter_context(tc.tile_pool(name="persist", bufs=1))
        stg16 = ctx.enter_context(tc.tile_pool(name="stg16", bufs=2))
        tmp32 = ctx.enter_context(tc.tile_pool(name="tmp32", bufs=2))
        ps = ctx.enter_context(tc.tile_pool(name="ps", bufs=3, space="PSUM"))
        pss = ctx.enter_context(tc.tile_pool(name="pss", bufs=1, space="PSUM"))

        def load_blocks(dst, src_ap):
            for t, (s, sz) in enumerate(KT):
                nc.sync.dma_start(dst[0:sz, _mblk(t)], src_ap[s:s + sz, :])

        def zero_pad(t32, eng=None):
            (eng or nc.vector).memset(t32[64:128, 640:960], 0.0)

        # ---- persistent tiles ----
        sr = [persist.tile([128, 960], F16, tag=f"sr{c}", name=f"sr{c}") for c in range(C)]
        si = [persist.tile([128, 960], F16, tag=f"si{c}", name=f"si{c}") for c in range(C)]
        mask2 = persist.tile([128, 960], F32, tag="mask2", name="mask2")
        mask1 = persist.tile([128, 960], F32, tag="mask1", name="mask1")
        p_re = persist.tile([128, 960], F32, tag="p_re", name="p_re")
        p_im = persist.tile([128, 960], F32, tag="p_im", name="p_im")
        r_re = persist.tile([128, 960], F32, tag="r_re", name="r_re")
        r_im = persist.tile([128, 960], F32, tag="r_im", name="r_im")
        x_re = persist.tile([128, 960], F32, tag="x_re", name="x_re")
        x_im = persist.tile([128, 960], F32, tag="x_im", name="x_im")
        ap_re = persist.tile([128, 960], F32, tag="ap_re", name="ap_re")
        ap_im = persist.tile([128, 960], F32, tag="ap_im", name="ap_im")
        fr = persist.tile([128, 960], F16, tag="fr", name="fr")
        fi = persist.tile([128, 960], F16, tag="fi", name="fi")
        nfi = persist.tile([128, 960], F16, tag="nfi", name="nfi")
        ones_col = persist.tile([128, 1], F32, tag="ones_col", name="ones_col")
        ones_row = persist.tile([1, 128], F32, tag="ones_row", name="ones_row")
        lam_b = persist.tile([128, 1], F32, tag="lam_b", name="lam_b")
        alpha_b = persist.tile([128, 1], F32, tag="alpha_b", name="alpha_b")
        beta_b = persist.tile([128, 1], F32, tag="beta_b", name="beta_b")
        dacc = persist.tile([128, 2], F32, tag="dacc", name="dacc")
        sc = persist.tile([1, 8], F32, tag="sc", name="sc")
        # sc slots: 0=rTr, 1=inv_rTr, 2=pAp, 3=alpha, 4=rTrNew, 5=beta, 6=tmp
        scr = persist.tile([128, 960], F32, tag="scr", name="scr")
        scr2 = persist.tile([128, 960], F32, tag="scr2", name="scr2")
        jnk = persist.tile([128, 960], F32, tag="jnk", name="jnk")

        def emit_side(zr16, zi16, chain, consume):
            """psum(re,im) per m-block of Z^T @ A, complex. consume(mt,msz,pre,pim)."""
            if chain == "F":
                mov = [(zr16, fr, "re"), (zr16, fi, "im"),
                       (zi16, nfi, "re"), (zi16, fr, "im")]
            else:  # G = conj(F)
                mov = [(zr16, fr, "re"), (zr16, nfi, "im"),
                       (zi16, fi, "re"), (zi16, fr, "im")]
            for mt, (ms, msz) in enumerate(KT):
                pre = ps.tile([128, 320], F32, tag="ps_re", name="ps_re", bufs=4)
                pim = ps.tile([128, 320], F32, tag="ps_im", name="ps_im")
                cnt = {"re": 0, "im": 0}
                for kt, (ks, ksz) in enumerate(KT):
                    for zi_idx, (z, a, dst) in enumerate(mov):
                        lo = 320 * kt + 128 * mt
                        pt = (pre if dst == "re" else pim)[0:msz, :]
                        cnt[dst] += 1
                        zt = z[0:ksz, lo:lo + msz]
                        at = a[0:ksz, _mblk(kt)]
                        nc.tensor.matmul(pt, zt, at, start=(cnt[dst] == 1),
                                         stop=(cnt[dst] == 6))
                consume(mt, msz, pre, pim)

        def cmul_to_fp16(ar, ai, br, bi, outr, outi):
            """(outr + i outi) = (ar + i ai)(br + i bi); fp32 in, fp16 out."""
            t1 = tmp32.tile([128, 960], F32, tag="mm_t1", name="mm_t1")
            t2 = tmp32.tile([128, 960], F32, tag="mm_t2", name="mm_t2")
            t3 = tmp32.tile([128, 960], F32, tag="mm_t3", name="mm_t3")
            t4 = tmp32.tile([128, 960], F32, tag="mm_t4", name="mm_t4")
            nc.gpsimd.tensor_mul(t1[:], ar[:], br[:])
            nc.gpsimd.tensor_mul(t2[:], ai[:], bi[:])
            nc.gpsimd.tensor_mul(t3[:], ar[:], bi[:])
            nc.gpsimd.tensor_mul(t4[:], ai[:], br[:])
            nc.vector.tensor_sub(outr[:], t1[:], t2[:])
            nc.vector.tensor_add(outi[:], t3[:], t4[:])
            nc.sync.dma_start(outi[64:128, 640:960], outi[0:64, 640:960])

        def combine_coil(c, u4r, u4i):
            """ap += conj(s_c) * u4 (fp32)."""
            t1 = tmp32.tile([128, 960], F32, tag="mm_t1", name="mm_t1")
            t2 = tmp32.tile([128, 960], F32, tag="mm_t2", name="mm_t2")
            t3 = tmp32.tile([128, 960], F32, tag="mm_t3", name="mm_t3")
            t4 = tmp32.tile([128, 960], F32, tag="mm_t4", name="mm_t4")
            nc.vector.tensor_mul(t1[:], sr[c][:], u4r[:])
            nc.vector.tensor_mul(t2[:], si[c][:], u4i[:])
            nc.vector.tensor_mul(t3[:], sr[c][:], u4i[:])
            nc.vector.tensor_mul(t4[:], si[c][:], u4r[:])
            nc.vector.tensor_add(ap_re[:], ap_re[:], t1[:])
            nc.vector.tensor_add(ap_re[:], ap_re[:], t2[:])
            nc.vector.tensor_add(ap_im[:], ap_im[:], t3[:])
            nc.vector.tensor_sub(ap_im[:], ap_im[:], t4[:])

        def ifft_and_combine(c, inr16, ini16):
            """Emit S3/S4 G-chain; return deferred combine closure."""
            s3r = stg16.tile([128, 960], F16, tag="s3r", name="s3r")
            s3i = stg16.tile([128, 960], F16, tag="s3i", name="s3i")

            def consume3(mt, msz, pre, pim):
                nc.scalar.copy(s3r[0:msz, _mblk(mt)], pre[0:msz, :])
                nc.scalar.copy(s3i[0:msz, _mblk(mt)], pim[0:msz, :])
                if mt == 2:
                    nc.sync.dma_start(s3i[64:128, 640:960], s3i[0:64, 640:960])
            emit_side(inr16, ini16, "G", consume3)

            u4r = tmp32.tile([128, 960], F32, tag="u4r", name="u4r")
            u4i = tmp32.tile([128, 960], F32, tag="u4i", name="u4i")
            zero_pad(u4r, nc.gpsimd)
            zero_pad(u4i, nc.gpsimd)

            def consume4(mt, msz, pre, pim):
                nc.scalar.copy(u4r[0:msz, _mblk(mt)], pre[0:msz, :])
                nc.scalar.copy(u4i[0:msz, _mblk(mt)], pim[0:msz, :])
            emit_side(s3r, s3i, "G", consume4)
            return lambda: combine_coil(c, u4r, u4i)

        def dot_to_sc(a_re, b_re, a_im, b_im, slot):
            """sc[0, slot] = sum(a_re*b_re + a_im*b_im) over valid region.

            tensor_tensor_reduce faults on this hardware path, so: self-dots
            use ACT Square+accum_out; cross-dots DVE-mult + ACT Copy+accum.
            """
            SQ = mybir.ActivationFunctionType.Square
            CP = mybir.ActivationFunctionType.Copy
            if a_re is b_re and a_im is b_im:
                nc.scalar.activation(jnk[:], a_re[:], SQ, accum_out=dacc[:, 0:1])
                nc.scalar.activation(jnk[:], a_im[:], SQ, accum_out=dacc[:, 1:2])
            else:
                nc.vector.tensor_mul(scr[:], a_re[:], b_re[:])
                nc.vector.tensor_mul(scr2[:], a_im[:], b_im[:])
                nc.scalar.activation(jnk[:], scr[:], CP, accum_out=dacc[:, 0:1])
                nc.scalar.activation(jnk[:], scr2[:], CP, accum_out=dacc[:, 1:2])
            pd = pss.tile([1, 2], F32, tag="pdot", name="pdot")
            nc.tensor.matmul(pd[0:1, 0:2], ones_col[:, 0:1], dacc[:, 0:2],
                             start=True, stop=True)
            nc.vector.tensor_copy(sc[0:1, 6:8], pd[0:1, 0:2])
            nc.vector.tensor_add(sc[0:1, slot:slot + 1], sc[0:1, 6:7],
                                 sc[0:1, 7:8])

        # ---- load constants + inputs ----
        nc.sync.dma_start(fr[:], d["c_fr"].ap())
        nc.sync.dma_start(fi[:], d["c_fi"].ap())
        nc.sync.dma_start(nfi[:], d["c_nfi"].ap())
        nc.sync.dma_start(ones_col[:], d["ones_col"].ap())
        nc.sync.dma_start(ones_row[:], d["ones_row"].ap())
        nc.sync.dma_start(lam_b[:], d["lam_b"].ap())
        for c in range(C):
            load_blocks(sr[c], d["s_re"].ap()[c])
            load_blocks(si[c], d["s_im"].ap()[c])
            zero_pad(sr[c])
            zero_pad(si[c])
        load_blocks(mask1, d["mask"].ap())
        zero_pad(mask1)
        load_blocks(x_re, d["x_re"].ap())
        load_blocks(x_im, d["x_im"].ap())
        zero_pad(x_re)
        zero_pad(x_im)
        nc.vector.tensor_mul(mask2[:], mask1[:], mask1[:])

        nc.vector.memset(ap_re[:], 0.0)
        nc.vector.memset(ap_im[:], 0.0)

        # ---- phase 1: rhs ----
        def make_my(c):
            yr = tmp32.tile([128, 960], F32, tag="yr", name="yr")
            yi = tmp32.tile([128, 960], F32, tag="yi", name="yi")
            load_blocks(yr, d["y_re"].ap()[c])
            load_blocks(yi, d["y_im"].ap()[c])
            zero_pad(yr, nc.gpsimd)
            zero_pad(yi, nc.gpsimd)
            myr = stg16.tile([128, 960], F16, tag="spr", name="myr")
            myi = stg16.tile([128, 960], F16, tag="spi", name="myi")
            nc.vector.tensor_mul(myr[:], yr[:], mask1[:])
            nc.vector.tensor_mul(myi[:], yi[:], mask1[:])
            nc.sync.dma_start(myi[64:128, 640:960], myi[0:64, 640:960])
            return myr, myi

        my_next = make_my(0)
        pending = None
        for c in range(C):
            myr, myi = my_next
            if c + 1 < C:
                my_next = make_my(c + 1)
            comb = ifft_and_combine(c, myr, myi)
            if pending is not None:
                pending()
            pending = comb
        if pending is not None:
            pending()

        # r0 = p0 = rhs = ap + lam*x ; x0 = 0
        nc.vector.tensor_scalar_mul(scr[:], x_re[:], lam_b[:, 0:1])
        nc.vector.tensor_add(r_re[:], ap_re[:], scr[:])
        nc.vector.tensor_scalar_mul(scr2[:], x_im[:], lam_b[:, 0:1])
        nc.vector.tensor_add(r_im[:], ap_im[:], scr2[:])
        nc.scalar.copy(p_re[:], r_re[:])
        nc.scalar.copy(p_im[:], r_im[:])
        nc.vector.memset(x_re[:], 0.0)
        nc.vector.memset(x_im[:], 0.0)

        dot_to_sc(r_re, r_re, r_im, r_im, 0)          # rTr0
        nc.vector.reciprocal(sc[0:1, 1:2], sc[0:1, 0:1])

        # ---- phase 2: CG iterations ----
        def cg_iteration():
            nc.vector.tensor_scalar_mul(ap_re[:], p_re[:], lam_b[:, 0:1])
            nc.vector.tensor_scalar_mul(ap_im[:], p_im[:], lam_b[:, 0:1])

            def make_sp(c):
                spr = stg16.tile([128, 960], F16, tag="spr", name="spr")
                spi = stg16.tile([128, 960], F16, tag="spi", name="spi")
                cmul_to_fp16(sr[c], si[c], p_re, p_im, spr, spi)
                return spr, spi

            sp_next = make_sp(0)
            pending = None
            for c in range(C):
                spr, spi = sp_next
                s1r = stg16.tile([128, 960], F16, tag="s1r", name="s1r")
                s1i = stg16.tile([128, 960], F16, tag="s1i", name="s1i")

                def consume1(mt, msz, pre, pim):
                    nc.scalar.copy(s1r[0:msz, _mblk(mt)], pre[0:msz, :])
                    nc.scalar.copy(s1i[0:msz, _mblk(mt)], pim[0:msz, :])
                    if mt == 2:
                        nc.sync.dma_start(s1i[64:128, 640:960],
                                          s1i[0:64, 640:960])
                emit_side(spr, spi, "F", consume1)

                wr = stg16.tile([128, 960], F16, tag="wr", name="wr")
                wi = stg16.tile([128, 960], F16, tag="wi", name="wi")

                def consume2(mt, msz, pre, pim):
                    nc.vector.tensor_mul(wr[0:msz, _mblk(mt)], pre[0:msz, :],
                                         mask2[0:msz, _mblk(mt)])
                    nc.vector.tensor_mul(wi[0:msz, _mblk(mt)], pim[0:msz, :],
                                         mask2[0:msz, _mblk(mt)])
                    if mt == 2:
                        nc.sync.dma_start(wi[64:128, 640:960],
                                          wi[0:64, 640:960])
                emit_side(s1r, s1i, "F", consume2)
                if pending is not None:
                    pending()

                # prepare next coil's SP before this coil's ifft+combine so the
                # DVE/GpSimd streams feed the PE ahead of the combine chain
                if c + 1 < C:
                    sp_next = make_sp(c + 1)
                pending = ifft_and_combine(c, wr, wi)
            pending()

            dot_to_sc(p_re, ap_re, p_im, ap_im, 2)    # pAp
            nc.vector.reciprocal(sc[0:1, 6:7], sc[0:1, 2:3])
            nc.vector.tensor_mul(sc[0:1, 3:4], sc[0:1, 0:1], sc[0:1, 6:7])  # alpha
            pb = pss.tile([128, 1], F32, tag="pdot", name="pbc")
            nc.tensor.matmul(pb[:, 0:1], ones_row[0:1, :], sc[0:1, 3:4],
                             start=True, stop=True)
            nc.scalar.copy(alpha_b[:, 0:1], pb[:, 0:1])

            nc.vector.tensor_scalar_mul(scr[:], ap_re[:], alpha_b[:, 0:1])
            nc.vector.tensor_sub(r_re[:], r_re[:], scr[:])
            nc.vector.tensor_scalar_mul(scr2[:], ap_im[:], alpha_b[:, 0:1])
            nc.vector.tensor_sub(r_im[:], r_im[:], scr2[:])

            dot_to_sc(r_re, r_re, r_im, r_im, 4)      # rTrNew
            nc.vector.tensor_mul(sc[0:1, 5:6], sc[0:1, 4:5], sc[0:1, 1:2])  # beta
            nc.vector.tensor_copy(sc[0:1, 0:1], sc[0:1, 4:5])
            nc.vector.reciprocal(sc[0:1, 1:2], sc[0:1, 4:5])
            pb2 = pss.tile([128, 1], F32, tag="pdot", name="pbc2")
            nc.tensor.matmul(pb2[:, 0:1], ones_row[0:1, :], sc[0:1, 5:6],
                             start=True, stop=True)
            nc.scalar.copy(beta_b[:, 0:1], pb2[:, 0:1])

            nc.scalar.mul(scr[:], p_re[:], beta_b[:, 0:1])
            nc.scalar.mul(scr2[:], p_im[:], beta_b[:, 0:1])
            nc.vector.tensor_scalar(jnk[:], p_re[:], alpha_b[:, 0:1], None, MULT)
            nc.vector.tensor_add(x_re[:], x_re[:], jnk[:])
            nc.vector.tensor_add(p_re[:], r_re[:], scr[:])
            nc.vector.tensor_scalar(scr[:], p_im[:], alpha_b[:, 0:1], None, MULT)
            nc.vector.tensor_add(p_im[:], r_im[:], scr2[:])
            nc.vector.tensor_add(x_im[:], x_im[:], scr[:])

        if UNROLL:
            for _ in range(N_ITER):
                cg_iteration()
        else:
            with tc.For_i(0, N_ITER, 1):
                cg_iteration()

        for t, (s, sz) in enumerate(KT):
            nc.sync.dma_start(d["out"].ap()[0, s:s + sz, :], x_re[0:sz, _mblk(t)])
            nc.sync.dma_start(d["out"].ap()[1, s:s + sz, :], x_im[0:sz, _mblk(t)])

    nc.compile()
    return nc


def kernel(lambdaa, x_re, x_im, y_re, y_im, smaps_re, smaps_im, mask):
    global _PROGRAM
    lambdaa = np.asarray(lambdaa, np.float32)
    arrs = {
        "x_re": x_re, "x_im": x_im, "y_re": y_re, "y_im": y_im,
    }
    arrs = {k: np.ascontiguousarray(np.asarray(v, np.float32))
            for k, v in arrs.items()}
    arrs["s_re"] = np.ascontiguousarray(np.asarray(smaps_re, np.float16))
    arrs["s_im"] = np.ascontiguousarray(np.asarray(smaps_im, np.float16))
    mask = np.ascontiguousarray(np.asarray(mask, np.float32))

    if _PROGRAM is None:
        _PROGRAM = _build_program()
    nc = _PROGRAM

    consts = _build_consts()
    lam_b = np.full((128, 1), float(lambdaa[0]), np.float32)
    in_maps = []
    for i in range(B):
        in_maps.append({
            **{k: v[i] for k, v in arrs.items()},
            "mask": np.ascontiguousarray(mask[i, 0]),
            "lam_b": lam_b,
            **consts,
        })

    res = bass_utils.run_bass_kernel_spmd(nc, in_maps, core_ids=list(range(B)),
                                          trace=TRACE)
    kernel._last_result = res
    out = np.empty((B, H, H, 2), np.float32)
    for i in range(B):
        o = res.results[i]["out"]
        out[i, :, :, 0] = o[0]
        out[i, :, :, 1] = o[1]
    return out


# revision 26
# speedup vs baseline: 1.5409x; 1.0623x over previous
"""CG-SENSE MRI reconstruction (nn_CGClass) on 8 Trainium2 NeuronCores.

Strategy: data-parallel over batch (B=8 -> 1 sample per core). Per core the
whole CG (10 iterations) runs on-chip. fft2/ifft2 are dense 320-point DFTs
done on the tensor engine as fp16 matmuls via the transpose-free primitive
OUT = Z^T @ A (data stationary, DFT matrix moving); applying it twice gives
F @ Z @ F with no transposes. CG state stays fp32; only matmul operands are
fp16 (measured end-to-end error ~3e-4, reference tol is far looser).

Layout: each 320x320 real array lives in SBUF as [128, 960]: free-dim block
t in {0,1,2} holds image rows [128t : 128t+{128,128,64}]. Block 2 uses
partitions 0..63; pad regions are kept zero (NaN hygiene for reductions).

The ragged K=64 block-2 contraction tail is packed: each imag tile's block 2
is DMA'd into the matching real tile's partitions 64:127, so one K=128
matmul with a host-stacked [table_for_re; table_for_im] moving operand
covers both 64-row tails -> 10 matmuls per m-block instead of 12.
"""
import os
from contextlib import ExitStack

import numpy as np

import concourse.bass as bass  # noqa: F401
import concourse.tile as tile
from concourse import mybir, bass_utils, bacc

F32 = mybir.dt.float32
F16 = mybir.dt.float16
MULT = mybir.AluOpType.mult
ADD = mybir.AluOpType.add

H = 320
B, C = 8, 12
N_ITER = int(os.environ.get("CG_ITERS", "10"))
KT = [(0, 128), (128, 128), (256, 64)]  # (row_start, rows) per block

_PROGRAM = None
TRACE = bool(os.environ.get("CG_TRACE"))
UNROLL = bool(os.environ.get("CG_UNROLL"))
PACK = os.environ.get("CG_PACK", "1") != "0"


def _mblk(t):
    return slice(320 * t, 320 * t + 320)


def _build_consts():
    j = np.arange(H)
    ang = -2.0 * np.pi * np.outer(j, j) / H
    scale = 1.0 / np.sqrt(H)
    Fr = (np.cos(ang) * scale).astype(np.float32)
    Fi = (np.sin(ang) * scale).astype(np.float32)

    def blocks(a):
        out = np.zeros((128, 960), np.float32)
        for t, (s, sz) in enumerate(KT):
            out[:sz, 320 * t:320 * t + 320] = a[s:s + sz]
        # block-2 rows duplicated at partitions 64..127 for row-group-packed
        # K=64 matmuls (stationary pairs at array rows 0-63 / 64-127)
        out[64:128, 640:960] = out[0:64, 640:960]
        return out

    def pk(ta, tb):
        out = np.zeros((128, 320), np.float32)
        out[0:64] = ta[256:320]
        out[64:128] = tb[256:320]
        return out

    return {
        "c_fr": blocks(Fr).astype(np.float16),
        "c_fi": blocks(Fi).astype(np.float16),
        "c_nfi": blocks(-Fi).astype(np.float16),
        # packed block-2 moving tables: [table_for_zr rows 256:320 ;
        # table_for_zi rows 256:320] per chain / component
        "pk_f_re": pk(Fr, -Fi).astype(np.float16),
        "pk_f_im": pk(Fi, Fr).astype(np.float16),
        "pk_g_re": pk(Fr, Fi).astype(np.float16),
        "pk_g_im": pk(-Fi, Fr).astype(np.float16),
        "ones_col": np.ones((128, 1), np.float32),
        "ones_row": np.ones((1, 128), np.float32),
    }


def _build_program():
    nc = bacc.Bacc("TRN2", target_bir_lowering=False, debug=False)

    d = {}
    d["x_re"] = nc.dram_tensor("x_re", [H, H], F32, kind="ExternalInput")
    d["x_im"] = nc.dram_tensor("x_im", [H, H], F32, kind="ExternalInput")
    d["y_re"] = nc.dram_tensor("y_re", [C, H, H], F32, kind="ExternalInput")
    d["y_im"] = nc.dram_tensor("y_im", [C, H, H], F32, kind="ExternalInput")
    d["s_re"] = nc.dram_tensor("s_re", [C, H, H], F16, kind="ExternalInput")
    d["s_im"] = nc.dram_tensor("s_im", [C, H, H], F16, kind="ExternalInput")
    d["mask"] = nc.dram_tensor("mask", [H, H], F32, kind="ExternalInput")
    d["lam_b"] = nc.dram_tensor("lam_b", [128, 1], F32, kind="ExternalInput")
    d["c_fr"] = nc.dram_tensor("c_fr", [128, 960], F16, kind="ExternalInput")
    d["c_fi"] = nc.dram_tensor("c_fi", [128, 960], F16, kind="ExternalInput")
    d["c_nfi"] = nc.dram_tensor("c_nfi", [128, 960], F16, kind="ExternalInput")
    d["pk_f_re"] = nc.dram_tensor("pk_f_re", [128, 320], F16, kind="ExternalInput")
    d["pk_f_im"] = nc.dram_tensor("pk_f_im", [128, 320], F16, kind="ExternalInput")
    d["pk_g_re"] = nc.dram_tensor("pk_g_re", [128, 320], F16, kind="ExternalInput")
    d["pk_g_im"] = nc.dram_tensor("pk_g_im", [128, 320], F16, kind="ExternalInput")
    d["ones_col"] = nc.dram_tensor("ones_col", [128, 1], F32, kind="ExternalInput")
    d["ones_row"] = nc.dram_tensor("ones_row", [1, 128], F32, kind="ExternalInput")
    d["out"] = nc.dram_tensor("out", [2, H, H], F32, kind="ExternalOutput")

    with tile.TileContext(nc) as tc, ExitStack() as ctx:
        persist = ctx.enter_context(tc.tile_pool(name="persist", bufs=1))
        stg16 = ctx.enter_context(tc.tile_pool(name="stg16", bufs=2))
        tmp32 = ctx.enter_context(tc.tile_pool(name="tmp32", bufs=2))
        ps = ctx.enter_context(tc.tile_pool(name="ps", bufs=3, space="PSUM"))
        pss = ctx.enter_context(tc.tile_pool(name="pss", bufs=1, space="PSUM"))

        def load_blocks(dst, src_ap):
            for t, (s, sz) in enumerate(KT):
                nc.sync.dma_start(dst[0:sz, _mblk(t)], src_ap[s:s + sz, :])

        def zero_pad(t32, eng=None):
            (eng or nc.vector).memset(t32[64:128, 640:960], 0.0)

        # ---- persistent tiles ----
        sr = [persist.tile([128, 960], F16, tag=f"sr{c}", name=f"sr{c}") for c in range(C)]
        si = [persist.tile([128, 960], F16, tag=f"si{c}", name=f"si{c}") for c in range(C)]
        mask2 = persist.tile([128, 960], F32, tag="mask2", name="mask2")
        mask1 = persist.tile([128, 960], F32, tag="mask1", name="mask1")
        p_re = persist.tile([128, 960], F32, tag="p_re", name="p_re")
        p_im = persist.tile([128, 960], F32, tag="p_im", name="p_im")
        r_re = persist.tile([128, 960], F32, tag="r_re", name="r_re")
        r_im = persist.tile([128, 960], F32, tag="r_im", name="r_im")
        x_re = persist.tile([128, 960], F32, tag="x_re", name="x_re")
        x_im = persist.tile([128, 960], F32, tag="x_im", name="x_im")
        ap_re = persist.tile([128, 960], F32, tag="ap_re", name="ap_re")
        ap_im = persist.tile([128, 960], F32, tag="ap_im", name="ap_im")
        fr = persist.tile([128, 960], F16, tag="fr", name="fr")
        fi = persist.tile([128, 960], F16, tag="fi", name="fi")
        nfi = persist.tile([128, 960], F16, tag="nfi", name="nfi")
        pk_f_re = persist.tile([128, 320], F16, tag="pk_f_re", name="pk_f_re")
        pk_f_im = persist.tile([128, 320], F16, tag="pk_f_im", name="pk_f_im")
        pk_g_re = persist.tile([128, 320], F16, tag="pk_g_re", name="pk_g_re")
        pk_g_im = persist.tile([128, 320], F16, tag="pk_g_im", name="pk_g_im")
        ones_col = persist.tile([128, 1], F32, tag="ones_col", name="ones_col")
        ones_row = persist.tile([1, 128], F32, tag="ones_row", name="ones_row")
        lam_b = persist.tile([128, 1], F32, tag="lam_b", name="lam_b")
        alpha_b = persist.tile([128, 1], F32, tag="alpha_b", name="alpha_b")
        beta_b = persist.tile([128, 1], F32, tag="beta_b", name="beta_b")
        dacc = persist.tile([128, 2], F32, tag="dacc", name="dacc")
        sc = persist.tile([1, 8], F32, tag="sc", name="sc")
        # sc slots: 0=rTr, 1=inv_rTr, 2=pAp, 3=alpha, 4=rTrNew, 5=beta, 6=tmp
        scr = persist.tile([128, 960], F32, tag="scr", name="scr")
        scr2 = persist.tile([128, 960], F32, tag="scr2", name="scr2")
        jnk = persist.tile([128, 960], F32, tag="jnk", name="jnk")
        jnk2 = persist.tile([128, 960], F32, tag="jnk2", name="jnk2")

        def emit_side(zr16, zi16, chain, consume, pack_out=False):
            """psum(re,im) per m-block of Z^T @ A, complex. consume(mt,msz,pre,pim).

            With PACK, zr16's partitions 64:127 @ cols 640:960 must hold
            zi16's block 2 (producers dup via DMA); the block-2 tails of the
            zr and zi contractions then merge into one K=128 matmul against
            the host-stacked pk tables."""
            if chain == "F":
                tB, tC = fi, nfi
                pkre, pkim = pk_f_re, pk_f_im
            else:  # G = conj(F)
                tB, tC = nfi, fi
                pkre, pkim = pk_g_re, pk_g_im
            mm = nc.tensor.matmul
            for mt, (ms, msz) in enumerate(KT):
                pre = ps.tile([128, 320], F32, tag="ps_re", name="ps_re", bufs=4)
                pim = ps.tile([128, 320], F32, tag="ps_im", name="ps_im")
                prs = pre[0:msz, :]
                if pack_out and mt == 2:
                    pis = pim[64:128, :]
                else:
                    pis = pim[0:msz, :]
                if PACK:
                    zr0 = zr16[0:128, 128 * mt:128 * mt + msz]
                    zr1 = zr16[0:128, 320 + 128 * mt:320 + 128 * mt + msz]
                    zpk = zr16[0:128, 640 + 128 * mt:640 + 128 * mt + msz]
                    zi0 = zi16[0:128, 128 * mt:128 * mt + msz]
                    zi1 = zi16[0:128, 320 + 128 * mt:320 + 128 * mt + msz]
                    mm(prs, zr0, fr[:, _mblk(0)], start=True, stop=False)
                    mm(pis, zr0, tB[:, _mblk(0)], start=True, stop=False)
                    mm(prs, zr1, fr[:, _mblk(1)], start=False, stop=False)
                    mm(pis, zr1, tB[:, _mblk(1)], start=False, stop=False)
                    mm(prs, zpk, pkre[:, 0:320], start=False, stop=False)
                    mm(pis, zpk, pkim[:, 0:320], start=False, stop=False)
                    mm(prs, zi0, tC[:, _mblk(0)], start=False, stop=False)
                    mm(pis, zi0, fr[:, _mblk(0)], start=False, stop=False)
                    mm(prs, zi1, tC[:, _mblk(1)], start=False, stop=True)
                    mm(pis, zi1, fr[:, _mblk(1)], start=False, stop=True)
                else:
                    if chain == "F":
                        mov = [(zr16, fr, "re"), (zr16, fi, "im"),
                               (zi16, nfi, "re"), (zi16, fr, "im")]
                    else:
                        mov = [(zr16, fr, "re"), (zr16, nfi, "im"),
                               (zi16, fi, "re"), (zi16, fr, "im")]
                    cnt = {"re": 0, "im": 0}
                    for kt, (ks, ksz) in enumerate(KT):
                        for z, a, dst in mov:
                            lo = 320 * kt + 128 * mt
                            pt = prs if dst == "re" else pis
                            cnt[dst] += 1
                            zt = z[0:ksz, lo:lo + msz]
                            at = a[0:ksz, _mblk(kt)]
                            mm(pt, zt, at, start=(cnt[dst] == 1),
                               stop=(cnt[dst] == 6))
                consume(mt, msz, prs, pis)

        def cmul_to_fp16(ar, ai, br, bi, outr, outi):
            """(outr + i outi) = (ar + i ai)(br + i bi); fp32 in, fp16 out."""
            t1 = tmp32.tile([128, 960], F32, tag="mm_t1", name="mm_t1")
            t2 = tmp32.tile([128, 960], F32, tag="mm_t2", name="mm_t2")
            t3 = tmp32.tile([128, 960], F32, tag="mm_t3", name="mm_t3")
            t4 = tmp32.tile([128, 960], F32, tag="mm_t4", name="mm_t4")
            nc.gpsimd.tensor_mul(t1[:], ar[:], br[:])
            nc.gpsimd.tensor_mul(t2[:], ai[:], bi[:])
            nc.vector.tensor_mul(t3[:], ar[:], bi[:])
            nc.vector.tensor_mul(t4[:], ai[:], br[:])
            nc.vector.tensor_sub(outr[:], t1[:], t2[:])
            nc.vector.tensor_add(outi[:], t3[:], t4[:])
            nc.sync.dma_start(outr[64:128, 640:960], outi[0:64, 640:960])

        def combine_coil(c, u4r, u4i):
            """ap += conj(s_c) * u4 (fp32)."""
            t1 = tmp32.tile([128, 960], F32, tag="mm_t1", name="mm_t1")
            t2 = tmp32.tile([128, 960], F32, tag="mm_t2", name="mm_t2")
            t3 = tmp32.tile([128, 960], F32, tag="mm_t3", name="mm_t3")
            t4 = tmp32.tile([128, 960], F32, tag="mm_t4", name="mm_t4")
            nc.vector.tensor_mul(t1[:], sr[c][:], u4r[:])
            nc.vector.tensor_mul(t2[:], si[c][:], u4i[:])
            nc.vector.tensor_mul(t3[:], sr[c][:], u4i[:])
            nc.vector.tensor_mul(t4[:], si[c][:], u4r[:])
            nc.vector.tensor_add(ap_re[:], ap_re[:], t1[:])
            nc.vector.tensor_add(ap_re[:], ap_re[:], t2[:])
            nc.vector.tensor_add(ap_im[:], ap_im[:], t3[:])
            nc.vector.tensor_sub(ap_im[:], ap_im[:], t4[:])

        def ifft_and_combine(c, inr16, ini16):
            """Emit S3/S4 G-chain; return deferred combine closure."""
            s3r = stg16.tile([128, 960], F16, tag="s3r", name="s3r")
            s3i = stg16.tile([128, 960], F16, tag="s3i", name="s3i")

            def consume3(mt, msz, pre, pim):
                nc.scalar.copy(s3r[0:msz, _mblk(mt)], pre)
                if mt == 2:
                    nc.scalar.copy(s3r[64:128, 640:960], pim)
                else:
                    nc.scalar.copy(s3i[0:msz, _mblk(mt)], pim)
            emit_side(inr16, ini16, "G", consume3, pack_out=True)

            u4r = tmp32.tile([128, 960], F32, tag="u4r", name="u4r")
            u4i = tmp32.tile([128, 960], F32, tag="u4i", name="u4i")
            zero_pad(u4r, nc.gpsimd)
            zero_pad(u4i, nc.gpsimd)

            def consume4(mt, msz, pre, pim):
                nc.scalar.copy(u4r[0:msz, _mblk(mt)], pre)
                nc.scalar.copy(u4i[0:msz, _mblk(mt)], pim)
            emit_side(s3r, s3i, "G", consume4)
            return lambda: combine_coil(c, u4r, u4i)

        def dot_to_sc(a_re, b_re, a_im, b_im, slot):
            """sc[0, slot] = sum(a_re*b_re + a_im*b_im) over valid region.

            tensor_tensor_reduce faults on this hardware path, so: self-dots
            use ACT Square+accum_out; cross-dots DVE-mult + ACT Copy+accum.
            """
            SQ = mybir.ActivationFunctionType.Square
            CP = mybir.ActivationFunctionType.Copy
            if a_re is b_re and a_im is b_im:
                nc.scalar.activation(jnk[:], a_re[:], SQ, accum_out=dacc[:, 0:1])
                nc.scalar.activation(jnk[:], a_im[:], SQ, accum_out=dacc[:, 1:2])
            else:
                nc.vector.tensor_mul(scr[:], a_re[:], b_re[:])
                nc.vector.tensor_mul(scr2[:], a_im[:], b_im[:])
                nc.scalar.activation(jnk[:], scr[:], CP, accum_out=dacc[:, 0:1])
                nc.scalar.activation(jnk[:], scr2[:], CP, accum_out=dacc[:, 1:2])
            pd = pss.tile([1, 2], F32, tag="pdot", name="pdot")
            nc.tensor.matmul(pd[0:1, 0:2], ones_col[:, 0:1], dacc[:, 0:2],
                             start=True, stop=True)
            nc.vector.tensor_copy(sc[0:1, 6:8], pd[0:1, 0:2])
            nc.vector.tensor_add(sc[0:1, slot:slot + 1], sc[0:1, 6:7],
                                 sc[0:1, 7:8])

        # ---- load constants + inputs ----
        nc.sync.dma_start(fr[:], d["c_fr"].ap())
        nc.sync.dma_start(fi[:], d["c_fi"].ap())
        nc.sync.dma_start(nfi[:], d["c_nfi"].ap())
        nc.sync.dma_start(pk_f_re[:], d["pk_f_re"].ap())
        nc.sync.dma_start(pk_f_im[:], d["pk_f_im"].ap())
        nc.sync.dma_start(pk_g_re[:], d["pk_g_re"].ap())
        nc.sync.dma_start(pk_g_im[:], d["pk_g_im"].ap())
        nc.sync.dma_start(ones_col[:], d["ones_col"].ap())
        nc.sync.dma_start(ones_row[:], d["ones_row"].ap())
        nc.sync.dma_start(lam_b[:], d["lam_b"].ap())
        for c in range(C):
            load_blocks(sr[c], d["s_re"].ap()[c])
            load_blocks(si[c], d["s_im"].ap()[c])
            zero_pad(sr[c])
            zero_pad(si[c])
        load_blocks(mask1, d["mask"].ap())
        zero_pad(mask1)
        load_blocks(x_re, d["x_re"].ap())
        load_blocks(x_im, d["x_im"].ap())
        zero_pad(x_re)
        zero_pad(x_im)
        nc.vector.tensor_mul(mask2[:], mask1[:], mask1[:])
        nc.sync.dma_start(mask2[64:128, 640:960], mask2[0:64, 640:960])

        nc.vector.memset(ap_re[:], 0.0)
        nc.vector.memset(ap_im[:], 0.0)

        # ---- phase 1: rhs ----
        def make_my(c):
            yr = tmp32.tile([128, 960], F32, tag="yr", name="yr")
            yi = tmp32.tile([128, 960], F32, tag="yi", name="yi")
            load_blocks(yr, d["y_re"].ap()[c])
            load_blocks(yi, d["y_im"].ap()[c])
            zero_pad(yr, nc.gpsimd)
            zero_pad(yi, nc.gpsimd)
            myr = stg16.tile([128, 960], F16, tag="spr", name="myr")
            myi = stg16.tile([128, 960], F16, tag="spi", name="myi")
            nc.vector.tensor_mul(myr[:], yr[:], mask1[:])
            nc.vector.tensor_mul(myi[:], yi[:], mask1[:])
            nc.sync.dma_start(myr[64:128, 640:960], myi[0:64, 640:960])
            return myr, myi

        my_next = make_my(0)
        pending = None
        for c in range(C):
            myr, myi = my_next
            if c + 1 < C:
                my_next = make_my(c + 1)
            comb = ifft_and_combine(c, myr, myi)
            if pending is not None:
                pending()
            pending = comb
        if pending is not None:
            pending()

        # r0 = p0 = rhs = ap + lam*x ; x0 = 0
        nc.vector.tensor_scalar_mul(scr[:], x_re[:], lam_b[:, 0:1])
        nc.vector.tensor_add(r_re[:], ap_re[:], scr[:])
        nc.vector.tensor_scalar_mul(scr2[:], x_im[:], lam_b[:, 0:1])
        nc.vector.tensor_add(r_im[:], ap_im[:], scr2[:])
        nc.scalar.copy(p_re[:], r_re[:])
        nc.scalar.copy(p_im[:], r_im[:])
        nc.vector.memset(x_re[:], 0.0)
        nc.vector.memset(x_im[:], 0.0)

        dot_to_sc(r_re, r_re, r_im, r_im, 0)          # rTr0
        nc.vector.reciprocal(sc[0:1, 1:2], sc[0:1, 0:1])

        # ---- phase 2: CG iterations ----
        def cg_iteration():
            nc.vector.tensor_scalar_mul(ap_re[:], p_re[:], lam_b[:, 0:1])
            nc.vector.tensor_scalar_mul(ap_im[:], p_im[:], lam_b[:, 0:1])

            def make_sp(c):
                spr = stg16.tile([128, 960], F16, tag="spr", name="spr")
                spi = stg16.tile([128, 960], F16, tag="spi", name="spi")
                cmul_to_fp16(sr[c], si[c], p_re, p_im, spr, spi)
                return spr, spi

            sp_next = make_sp(0)
            pending = None
            for c in range(C):
                spr, spi = sp_next
                s1r = stg16.tile([128, 960], F16, tag="s1r", name="s1r")
                s1i = stg16.tile([128, 960], F16, tag="s1i", name="s1i")

                def consume1(mt, msz, pre, pim):
                    nc.scalar.copy(s1r[0:msz, _mblk(mt)], pre)
                    if mt == 2:
                        nc.scalar.copy(s1r[64:128, 640:960], pim)
                    else:
                        nc.scalar.copy(s1i[0:msz, _mblk(mt)], pim)
                emit_side(spr, spi, "F", consume1, pack_out=True)

                wr = stg16.tile([128, 960], F16, tag="wr", name="wr")
                wi = stg16.tile([128, 960], F16, tag="wi", name="wi")

                def consume2(mt, msz, pre, pim):
                    nc.vector.tensor_mul(wr[0:msz, _mblk(mt)], pre,
                                         mask2[0:msz, _mblk(mt)])
                    if mt == 2:
                        nc.vector.tensor_mul(wr[64:128, 640:960], pim,
                                             mask2[64:128, 640:960])
                    else:
                        nc.vector.tensor_mul(wi[0:msz, _mblk(mt)], pim,
                                             mask2[0:msz, _mblk(mt)])
                emit_side(s1r, s1i, "F", consume2, pack_out=True)
                if pending is not None:
                    pending()

                # prepare next coil's SP before this coil's ifft+combine so the
                # DVE/GpSimd streams feed the PE ahead of the combine chain
                if c + 1 < C:
                    sp_next = make_sp(c + 1)
                pending = ifft_and_combine(c, wr, wi)
            pending()

            dot_to_sc(p_re, ap_re, p_im, ap_im, 2)    # pAp
            nc.vector.reciprocal(sc[0:1, 6:7], sc[0:1, 2:3])
            nc.vector.tensor_mul(sc[0:1, 3:4], sc[0:1, 0:1], sc[0:1, 6:7])  # alpha
            pb = pss.tile([128, 1], F32, tag="pdot", name="pbc")
            nc.tensor.matmul(pb[:, 0:1], ones_row[0:1, :], sc[0:1, 3:4],
                             start=True, stop=True)
            nc.scalar.copy(alpha_b[:, 0:1], pb[:, 0:1])

            nc.vector.tensor_scalar_mul(scr[:], ap_re[:], alpha_b[:, 0:1])
            nc.vector.tensor_sub(r_re[:], r_re[:], scr[:])
            nc.vector.tensor_scalar_mul(scr2[:], ap_im[:], alpha_b[:, 0:1])
            nc.vector.tensor_sub(r_im[:], r_im[:], scr2[:])

            dot_to_sc(r_re, r_re, r_im, r_im, 4)      # rTrNew
            nc.vector.tensor_mul(sc[0:1, 5:6], sc[0:1, 4:5], sc[0:1, 1:2])  # beta
            nc.vector.tensor_copy(sc[0:1, 0:1], sc[0:1, 4:5])
            nc.vector.reciprocal(sc[0:1, 1:2], sc[0:1, 4:5])
            pb2 = pss.tile([128, 1], F32, tag="pdot", name="pbc2")
            nc.tensor.matmul(pb2[:, 0:1], ones_row[0:1, :], sc[0:1, 5:6],
                             start=True, stop=True)
            nc.scalar.copy(beta_b[:, 0:1], pb2[:, 0:1])

            nc.scalar.mul(scr[:], p_re[:], beta_b[:, 0:1])
            nc.scalar.mul(scr2[:], p_im[:], beta_b[:, 0:1])
            nc.gpsimd.tensor_scalar(jnk[:], p_re[:], alpha_b[:, 0:1], None, MULT)
            nc.gpsimd.tensor_add(x_re[:], x_re[:], jnk[:])
            nc.vector.tensor_add(p_re[:], r_re[:], scr[:])
            nc.gpsimd.tensor_scalar(jnk2[:], p_im[:], alpha_b[:, 0:1], None, MULT)
            nc.vector.tensor_add(p_im[:], r_im[:], scr2[:])
            nc.gpsimd.tensor_add(x_im[:], x_im[:], jnk2[:])

        if UNROLL:
            for _ in range(N_ITER):
                cg_iteration()
        else:
            with tc.For_i(0, N_ITER, 1):
                cg_iteration()

        for t, (s, sz) in enumerate(KT):
            nc.sync.dma_start(d["out"].ap()[0, s:s + sz, :], x_re[0:sz, _mblk(t)])
            nc.sync.dma_start(d["out"].ap()[1, s:s + sz, :], x_im[0:sz, _mblk(t)])

    nc.compile()
    return nc


def kernel(lambdaa, x_re, x_im, y_re, y_im, smaps_re, smaps_im, mask):
    global _PROGRAM
    lambdaa = np.asarray(lambdaa, np.float32)
    arrs = {
        "x_re": x_re, "x_im": x_im, "y_re": y_re, "y_im": y_im,
    }
    arrs = {k: np.ascontiguousarray(np.asarray(v, np.float32))
            for k, v in arrs.items()}
    arrs["s_re"] = np.ascontiguousarray(np.asarray(smaps_re, np.float16))
    arrs["s_im"] = np.ascontiguousarray(np.asarray(smaps_im, np.float16))
    mask = np.ascontiguousarray(np.asarray(mask, np.float32))

    if _PROGRAM is None:
        _PROGRAM = _build_program()
    nc = _PROGRAM

    consts = _build_consts()
    lam_b = np.full((128, 1), float(lambdaa[0]), np.float32)
    in_maps = []
    for i in range(B):
        in_maps.append({
            **{k: v[i] for k, v in arrs.items()},
            "mask": np.ascontiguousarray(mask[i, 0]),
            "lam_b": lam_b,
            **consts,
        })

    res = bass_utils.run_bass_kernel_spmd(nc, in_maps, core_ids=list(range(B)),
                                          trace=TRACE)
    kernel._last_result = res
    out = np.empty((B, H, H, 2), np.float32)
    for i in range(B):
        o = res.results[i]["out"]
        out[i, :, :, 0] = o[0]
        out[i, :, :, 1] = o[1]
    return out
